# revision 1
# baseline (speedup 1.0000x reference)
"""ColBERT-style late-interaction similarity kernel for Trainium2 (8 NeuronCores).

Computes, for inputs
    cand_rep  [B=8, NC=64, CL=32,  D=128] f32
    ctxt_rep  [B=8, NK=64, TL=128, D=128] f32
    mask_cand [B=8, NC=64, CL=32]  bool
    mask_ctxt [B=8, NK=64, TL=128] bool
the output
    out[b,q,k] = masked_mean_t( max_c( cand[b,q,c,:] . ctxt[b,k,t,:] ) )   # [8, 64, 64] f32

Sharding: data-parallel over batch B — core b handles batch element b.

Per-core device pipeline:
  - host pre-transposes cand/ctxt to [D, tokens] (bf16) so D=128 is the
    contraction (partition) dim for the PE.
  - for each k (64): 4 matmuls [128d,128t]^T x [128d,512qc] -> PSUM scores
    [128t, 2048qc] (4 banks, ping-pong between two 4-bank slots)
  - max over c (free-dim groups of 32): alternating k's go to
      (a) DVE tensor_reduce(max) straight from PSUM, or
      (b) ScalarE copy PSUM->SBUF(bf16), then GPSIMD tensor_reduce(max)
    so the three reduce-capable engines run in parallel.
  - mean over t (partition dim): one tiny PE matmul per k against a
    mask_ctxt/denom weight column -> out PSUM [64q, 64k] -> SBUF -> HBM.
"""

import numpy as np
import ml_dtypes

B = 8
NC = 64   # n_cand
NK = 64   # n_ctxt
CL = 32   # cand_len
TL = 128  # ctxt_len
D = 128
QC = NC * CL   # 2048
KT = NK * TL   # 8192
NCORES = 8
NEG = -99999.0

# Per-k reduce-path assignment: "dve" (reduce straight from PSUM),
# "act_tree" (ScalarE copy + DVE max tree), "act_gp_tree" (ScalarE copy +
# GPSIMD level-1 + DVE finish). 4 direct-DVE / 60 ScalarE+tree balances the
# two PSUM-draining engines (~111us each per the TRN2 errata cost tables;
# TimelineSim concurs at 137us vs 157us for all-DVE).
_N_DVE = 4
_DVE_KS = {int(i * (NK / _N_DVE)) for i in range(_N_DVE)}
PATHS = ["dve" if k in _DVE_KS else "act_tree" for k in range(NK)]

_CACHE = {}


def _build_nc():
    import concourse.mybir as mybir
    import concourse.tile as tile
    from concourse import bacc

    f32 = mybir.dt.float32
    bf16 = mybir.dt.bfloat16
    X = mybir.AxisListType.X
    MAX = mybir.AluOpType.max

    nc = bacc.Bacc("TRN2", target_bir_lowering=False, debug=False)

    candT_d = nc.dram_tensor("candT", [D, QC], bf16, kind="ExternalInput").ap()
    ctxtT_d = nc.dram_tensor("ctxtT", [D, KT], bf16, kind="ExternalInput").ap()
    w_d = nc.dram_tensor("wvec", [TL, NK], bf16, kind="ExternalInput").ap()
    out_d = nc.dram_tensor("out", [NC, NK], f32, kind="ExternalOutput").ap()

    KG = 8            # ctxt DMA chunks (k-groups) for pipelined start
    KPG = NK // KG    # k's per chunk

    with tile.TileContext(nc) as tc:
        with (
            tc.tile_pool(name="const", bufs=1) as const_pool,
            tc.tile_pool(name="ctxt", bufs=KG) as ctxt_pool,
            tc.tile_pool(name="maxs", bufs=NK) as maxs_pool,
            tc.tile_pool(name="scratch", bufs=2) as scratch_pool,
            tc.tile_pool(name="psum", bufs=2, space="PSUM") as psum_pool,
        ):
            cand_sb = const_pool.tile([D, QC], bf16, tag="cand")
            nc.sync.dma_start(cand_sb[:], candT_d[:, :])
            w_sb = const_pool.tile([TL, NK], bf16, tag="wvec")
            nc.sync.dma_start(w_sb[:], w_d[:, :])

            ctxt_tiles = []
            for g in range(KG):
                t = ctxt_pool.tile([D, KT // KG], bf16, tag="ctxt")
                nc.sync.dma_start(
                    t[:], ctxtT_d[:, g * (KT // KG):(g + 1) * (KT // KG)]
                )
                ctxt_tiles.append(t)

            maxs_tiles = []
            for k in range(NK):
                g, r = divmod(k, KPG)
                lhsT = ctxt_tiles[g][:, r * TL:(r + 1) * TL]

                ps = psum_pool.tile([TL, QC], f32, tag="scores")
                for j in range(4):
                    nc.tensor.matmul(
                        out=ps[:, j * 512:(j + 1) * 512],
                        lhsT=lhsT,
                        rhs=cand_sb[:, j * 512:(j + 1) * 512],
                        start=True,
                        stop=True,
                    )

                mx = maxs_pool.tile([TL, NC], bf16, tag="maxs")
                path = PATHS[k]
                if path == "dve":
                    # DVE: segmented max straight from PSUM
                    nc.vector.tensor_reduce(
                        out=mx[:],
                        in_=ps[:].rearrange("p (q c) -> p q c", c=CL),
                        axis=X,
                        op=MAX,
                    )
                else:
                    # ScalarE copies/casts PSUM -> SBUF bf16, then a max tree
                    sc = scratch_pool.tile([TL, QC], bf16, tag="scratch")
                    nc.scalar.copy(sc[:], ps[:])
                    sc3 = sc[:].rearrange("p (q c) -> p q c", c=CL)
                    t1 = scratch_pool.tile([TL, QC // 2], bf16, tag="tree1")
                    t13 = t1[:].rearrange("p (q c) -> p q c", c=CL // 2)
                    if path == "act_gp_tree":
                        # GPSIMD does level 1 of the tree
                        nc.gpsimd.tensor_tensor(
                            out=t13, in0=sc3[:, :, 0:16], in1=sc3[:, :, 16:32],
                            op=MAX,
                        )
                    else:  # "act_tree": DVE does level 1
                        nc.vector.tensor_tensor(
                            out=t13, in0=sc3[:, :, 0:16], in1=sc3[:, :, 16:32],
                            op=MAX,
                        )
                    t2 = scratch_pool.tile([TL, QC // 4], bf16, tag="tree2")
                    t23 = t2[:].rearrange("p (q c) -> p q c", c=CL // 4)
                    nc.vector.tensor_tensor(
                        out=t23, in0=t13[:, :, 0:8], in1=t13[:, :, 8:16], op=MAX
                    )
                    nc.vector.tensor_reduce(
                        out=mx[:], in_=t23, axis=X, op=MAX
                    )
                maxs_tiles.append(mx)

            # stage 2: masked mean over t via PE (contraction over partitions)
            out_ps = psum_pool.tile([NC, NK], f32, tag="scores")
            for k in range(NK):
                nc.tensor.matmul(
                    out=out_ps[:, k:k + 1],
                    lhsT=maxs_tiles[k][:],
                    rhs=w_sb[:, k:k + 1],
                    start=True,
                    stop=True,
                )

            out_sb = const_pool.tile([NC, NK], f32, tag="outsb")
            nc.vector.tensor_copy(out_sb[:], out_ps[:])
            nc.sync.dma_start(out_d[:, :], out_sb[:])

    nc.finalize()
    return nc


def _get_nc():
    if "nc" not in _CACHE:
        _CACHE["nc"] = _build_nc()
    return _CACHE["nc"]


def _make_in_maps(cand_rep, ctxt_rep, mask_ctxt):
    bf16 = ml_dtypes.bfloat16
    cand_bf = np.ascontiguousarray(
        cand_rep.astype(bf16).reshape(B, QC, D).transpose(0, 2, 1)
    )
    ctxt_bf = np.ascontiguousarray(
        ctxt_rep.astype(bf16).reshape(B, KT, D).transpose(0, 2, 1)
    )
    m = mask_ctxt.astype(np.float32)                  # [B, NK, TL]
    denom = m.sum(-1, keepdims=True)                  # [B, NK, 1]
    with np.errstate(divide="ignore", invalid="ignore"):
        wv = (m / denom).transpose(0, 2, 1)           # [B, TL, NK]
    wv = np.ascontiguousarray(wv.astype(bf16))
    return [
        {"candT": cand_bf[b], "ctxtT": ctxt_bf[b], "wvec": wv[b]}
        for b in range(B)
    ]


def _run_device(in_maps, trace=False):
    from concourse.bass_utils import run_bass_kernel_spmd

    nc = _get_nc()
    return run_bass_kernel_spmd(nc, in_maps, list(range(NCORES)), trace=trace)


def _numpy_reference(cand_rep, ctxt_rep, mask_cand, mask_ctxt):
    # General fallback (exact), only used when mask_cand isn't all ones.
    out = np.empty((B, NC, NK), np.float32)
    mc = mask_cand.astype(bool)
    mt = mask_ctxt.astype(np.float32)
    denom = mt.sum(-1)  # [B, NK]
    for b in range(B):
        c = cand_rep[b].reshape(QC, D).astype(np.float32)
        t = ctxt_rep[b].reshape(KT, D).astype(np.float32)
        s = c @ t.T  # [QC, KT]
        s = s.reshape(NC, CL, NK, TL)
        s = np.where(mc[b][:, :, None, None], s, NEG)
        smax = s.max(axis=1)  # [NC, NK, TL]
        out[b] = (smax * mt[b][None]).sum(-1) / denom[b][None]
    return out


def kernel(cand_rep, ctxt_rep, mask_cand, mask_ctxt):
    cand_rep = np.asarray(cand_rep, dtype=np.float32)
    ctxt_rep = np.asarray(ctxt_rep, dtype=np.float32)
    mask_cand = np.asarray(mask_cand).astype(bool)
    mask_ctxt = np.asarray(mask_ctxt).astype(bool)
    assert cand_rep.shape == (B, NC, CL, D)
    assert ctxt_rep.shape == (B, NK, TL, D)

    if not mask_cand.all():
        # Rare general case (never hit by the benchmark fill): exact numpy path.
        return _numpy_reference(cand_rep, ctxt_rep, mask_cand, mask_ctxt)

    in_maps = _make_in_maps(cand_rep, ctxt_rep, mask_ctxt)
    res = _run_device(in_maps)
    out = np.stack([res.results[b]["out"] for b in range(B)])  # [B, NC, NK]
    return out.astype(np.float32)



# revision 2
# speedup vs baseline: 1.1641x; 1.1641x over previous
"""ColBERT-style late-interaction similarity kernel for Trainium2 (8 NeuronCores).

Computes, for inputs
    cand_rep  [B=8, NC=64, CL=32,  D=128] f32
    ctxt_rep  [B=8, NK=64, TL=128, D=128] f32
    mask_cand [B=8, NC=64, CL=32]  bool
    mask_ctxt [B=8, NK=64, TL=128] bool
the output
    out[b,q,k] = masked_mean_t( max_c( cand[b,q,c,:] . ctxt[b,k,t,:] ) )   # [8, 64, 64] f32

Sharding: data-parallel over batch B - core b handles batch element b.

Per-core pipeline:
  - host pre-packs cand/ctxt to fp8e4 with D split in two 64-halves
    (DoubleRow layout): each matmul contracts 2x64=128 at 0.5 cyc/row, so
    the PE (~28us) stays off the critical path.
  - PSUM is one [128, 4096] f32 ring of four 2-bank slots; a k's two
    half-tiles always form an aligned 2048-col pair, and Act-drained tiles
    free banks at half-k granularity so the PE runs ahead of the drain
    engines (kills the refill bubble on the in-order queues).
  - drain + max over c=32 split between DVE and ScalarE by a path table.
    The BIR verifier allows at most ONE PSUM operand per vector op and
    GPSIMD has no legal elementwise max on TRN2, so the legal menu is:
      A: per-slot DVE tensor_reduce(max) straight from PSUM -> final maxs
      h: ScalarE copies the c-high half to SBUF bf16; DVE tensor_tensor
         fuses the c-low drain with the pairwise max (one PSUM operand,
         c-major bf16 out), then a 4-k-batched DVE max tree at 2x_1p
      e: ScalarE per-item full copy PSUM->SBUF bf16, 2-k-batched DVE tree
    DVE tree instructions are spliced between drains (FIFO engines: ready
    ops queued behind a not-yet-ready drain absorb dependency bubbles).
  - mean over t (partition dim): per-k PE matmul against mask_ctxt/denom
    weight column -> out PSUM [64q, 64k] -> SBUF -> HBM.
"""

import numpy as np
import ml_dtypes

B = 8
NC = 64   # n_cand
NK = 64   # n_ctxt
CL = 32   # cand_len
TL = 128  # ctxt_len
D = 128
QC = NC * CL   # 2048
NCORES = 8
NEG = -99999.0

# Path counts: 'A' = two per-slot DVE tensor_reduce straight from PSUM,
# 'h' = Act copies the c-high half, DVE fuses drain+max of the c-low half
# against it (one PSUM operand - verifier legal), then a DVE tree,
# 'e' = Act per-item full copy + batched DVE tt-tree.
_NA, _NH, _NE = 12, 16, 36
_POP = 1   # DVE tree instructions spliced in after each drain


def _interleave(counts, n):
    """Largest-remainder interleave of class labels across n slots."""
    acc = {p: 0.0 for p in counts}
    seq = []
    for _ in range(n):
        for p in counts:
            acc[p] += counts[p] / n
        pick = max(acc, key=lambda p: acc[p])
        acc[pick] -= 1.0
        seq.append(pick)
    return seq


def _build_paths(na=_NA, nh=_NH, ne=_NE):
    """Alternate DVE-heavy ('A') and Act-consuming ('h'/'e') tiles so both
    drain engines always have fresh PSUM work; spread 'h' among 'e' the
    same way. Tree batches pair by subsequence index, not adjacency.
    Start with e,e so ScalarE ramps early; end with A,A so the Act tail
    drains while DVE finishes."""
    assert na + nh + ne == 64
    ax = _interleave({"A": na - 2, "x": nh + ne}, 62) + ["A", "A"]
    xs = _interleave({"e": ne - 2, "h": nh}, nh + ne - 2)
    xs = ["e", "e"] + xs
    it = iter(xs)
    return [p if p == "A" else next(it) for p in ax]


PATHS = _build_paths()

_CACHE = {}


def _build_nc(paths=None, pop=_POP):
    import concourse.mybir as mybir
    import concourse.tile as tile
    from concourse import bacc

    if paths is None:
        paths = PATHS
    f32 = mybir.dt.float32
    bf16 = mybir.dt.bfloat16
    fp8 = mybir.dt.float8e4
    MAX = mybir.AluOpType.max
    X = mybir.AxisListType.X
    DR = mybir.MatmulPerfMode.DoubleRow

    nc = bacc.Bacc("TRN2", target_bir_lowering=False, debug=False)

    candT_d = nc.dram_tensor("candT", [64, 2 * QC], fp8, kind="ExternalInput").ap()
    ctxtT_d = nc.dram_tensor("ctxtT", [64, NK * 256], fp8, kind="ExternalInput").ap()
    w_d = nc.dram_tensor("wvec", [TL, NK], bf16, kind="ExternalInput").ap()
    out_d = nc.dram_tensor("out", [NC, NK], f32, kind="ExternalOutput").ap()

    # ctxt DMA chunk boundaries (in k): small first chunks so the first
    # matmuls start as soon as possible
    CHUNKS = [0, 2, 8, 16, 24, 32, 40, 48, 56, 64]

    A_ks = [k for k in range(NK) if paths[k] == "A"]
    h_ks = [k for k in range(NK) if paths[k] == "h"]
    e_ks = [k for k in range(NK) if paths[k] == "e"]

    def make_groups(ks, first, size):
        """Split ks into groups: a small first group (earlier tree start),
        then `size`-sized groups. Returns {k: (group_idx, member_idx)}."""
        groups = []
        i = 0
        if ks:
            groups.append(ks[:first])
            i = first
        while i < len(ks):
            groups.append(ks[i:i + size])
            i += size
        return ({k: (g, j) for g, grp in enumerate(groups) for j, k in enumerate(grp)},
                len(groups))

    h_group, n_hg = make_groups(h_ks, 2, 4)
    e_group, n_eg = make_groups(e_ks, 1, 2)
    h_last = {grp: max(k for k, (g, _) in h_group.items() if g == grp)
              for grp in range(n_hg)}
    e_last = {grp: max(k for k, (g, _) in e_group.items() if g == grp)
              for grp in range(n_eg)}

    with tile.TileContext(nc) as tc:
        with (
            tc.tile_pool(name="const", bufs=1) as const_pool,
            tc.tile_pool(name="ctxt", bufs=len(CHUNKS) - 1) as ctxt_pool,
            tc.tile_pool(name="hgrp", bufs=3) as hgrp_pool,
            tc.tile_pool(name="hbuf", bufs=4) as hbuf_pool,
            tc.tile_pool(name="ebuf", bufs=3) as ebuf_pool,
            tc.tile_pool(name="tscr", bufs=2) as tscr_pool,
            tc.tile_pool(name="hmax", bufs=max(n_hg, 1)) as hmax_pool,
            tc.tile_pool(name="kmax", bufs=max(len(A_ks), 1)) as kmax_pool,
            tc.tile_pool(name="emax", bufs=max(n_eg, 1)) as emax_pool,
            tc.tile_pool(name="psum", bufs=1, space="PSUM") as psum_pool,
        ):
            cand_sb = const_pool.tile([64, 2 * QC], fp8, tag="cand")
            ctxt_tiles = []

            def ctxt_dma(g):
                nk = CHUNKS[g + 1] - CHUNKS[g]
                t = ctxt_pool.tile([64, nk * 256], fp8, tag="ctxt", name="ctxt")
                nc.sync.dma_start(
                    t[:], ctxtT_d[:, CHUNKS[g] * 256:CHUNKS[g + 1] * 256])
                ctxt_tiles.append(t)

            def cand_dma(j):
                # halves along qc (both D-halves): item h only needs chunk h
                nc.sync.dma_start(
                    cand_sb[:].rearrange("p (two n) -> p two n", two=2)[
                        :, :, j * 1024:(j + 1) * 1024
                    ],
                    candT_d[:].rearrange("p (two n) -> p two n", two=2)[
                        :, :, j * 1024:(j + 1) * 1024
                    ],
                )

            # HWDGE issues serially (~625ns each): order for fastest start
            ctxt_dma(0)
            cand_dma(0)
            cand_dma(1)
            ctxt_dma(1)
            for g in range(2, len(CHUNKS) - 1):
                ctxt_dma(g)
            w_sb = const_pool.tile([TL, NK], bf16, tag="wvec")
            nc.sync.dma_start(w_sb[:], w_d[:, :])

            cand3 = cand_sb[:].rearrange("p (two n) -> p two n", two=2)

            # PSUM: one 8-bank ring; slot s = item (k*2+h) % 4 at cols
            # [s*1024, (s+1)*1024). A k's two items are always an aligned
            # contiguous 2048-col pair ((2k)%4 in {0, 2}).
            psmega = psum_pool.tile([TL, 4096], f32, tag="scores", name="psmega")

            h_state = {}
            e_state = {}
            maxs_ap = [None] * NK   # per-k [128t, 64q] bf16 view for stage 2
            pending = []            # (ready_after_k, closure) for DVE tree ops

            def h_tree(st):
                members = st["members"]
                nj = len(members)
                buf = st["buf"]
                amx = hmax_pool.tile([TL, 4 * NC], bf16, tag="hmax", name="hmax")
                t2 = tscr_pool.tile([TL, 2048], bf16, tag="at2", name="at2")
                t3 = tscr_pool.tile([TL, 1024], bf16, tag="at3", name="at3")
                t4 = tscr_pool.tile([TL, 512], bf16, tag="at4", name="at4")

                def v(tt, jsz):
                    return tt[:].rearrange("p (j c q) -> p j c q", j=jsz, q=NC)[:, 0:nj]

                b3, t23, t33, t43 = v(buf, 4), v(t2, 4), v(t3, 4), v(t4, 4)
                am3 = amx[:].rearrange("p (j c q) -> p j c q", j=4, q=NC)[:, 0:nj]
                ops = [
                    lambda: nc.vector.tensor_tensor(
                        out=t23, in0=b3[:, :, 0:8], in1=b3[:, :, 8:16], op=MAX),
                    lambda: nc.vector.tensor_tensor(
                        out=t33, in0=t23[:, :, 0:4], in1=t23[:, :, 4:8], op=MAX),
                    lambda: nc.vector.tensor_tensor(
                        out=t43, in0=t33[:, :, 0:2], in1=t33[:, :, 2:4], op=MAX),
                    lambda: nc.vector.tensor_tensor(
                        out=am3, in0=t43[:, :, 0:1], in1=t43[:, :, 1:2], op=MAX),
                ]
                for jj, kk in enumerate(members):
                    maxs_ap[kk] = amx[:, jj * NC:(jj + 1) * NC]
                return ops

            def e_tree(st):
                # DVE tt-tree over the Act-copied tiles (GPSIMD has no legal
                # elementwise max on TRN2).
                members = st["members"]
                nj = 2 * len(members)
                buf = st["buf"]
                emx = emax_pool.tile([TL, 2 * NC], bf16, tag="emax", name="emax")
                v1 = tscr_pool.tile([TL, 2048], bf16, tag="et1", name="et1")
                v2 = tscr_pool.tile([TL, 1024], bf16, tag="et2", name="et2")
                v3 = tscr_pool.tile([TL, 512], bf16, tag="et3", name="et3")
                v4 = tscr_pool.tile([TL, 256], bf16, tag="et4", name="et4")

                def v(tt, c):
                    return tt[:].rearrange("p (j q c) -> p j q c", j=4, c=c)[:, 0:nj]

                e4, v14, v24, v34, v44 = v(buf, CL), v(v1, 16), v(v2, 8), v(v3, 4), v(v4, 2)
                em4 = emx[:].rearrange("p (j q c) -> p j q c", j=4, c=1)[:, 0:nj]
                ops = [
                    lambda: nc.vector.tensor_tensor(
                        out=v14, in0=e4[:, :, :, 0:16], in1=e4[:, :, :, 16:32], op=MAX),
                    lambda: nc.vector.tensor_tensor(
                        out=v24, in0=v14[:, :, :, 0:8], in1=v14[:, :, :, 8:16], op=MAX),
                    lambda: nc.vector.tensor_tensor(
                        out=v34, in0=v24[:, :, :, 0:4], in1=v24[:, :, :, 4:8], op=MAX),
                    lambda: nc.vector.tensor_tensor(
                        out=v44, in0=v34[:, :, :, 0:2], in1=v34[:, :, :, 2:4], op=MAX),
                    lambda: nc.vector.tensor_tensor(
                        out=em4, in0=v44[:, :, :, 0:1], in1=v44[:, :, :, 1:2], op=MAX),
                ]
                for jj, kk in enumerate(members):
                    maxs_ap[kk] = emx[:, jj * NC:(jj + 1) * NC]
                return ops

            import bisect
            for k in range(NK):
                g = bisect.bisect_right(CHUNKS, k) - 1
                r = k - CHUNKS[g]
                lhsT = ctxt_tiles[g][:, r * 256:(r + 1) * 256].rearrange(
                    "p (two m) -> p two m", two=2
                )

                s0 = (2 * k) % 4          # k's aligned slot pair: s0, s0+1
                off = s0 * 1024
                path = paths[k]

                if path == "A":
                    # 4 matmuls, then one DVE tensor_reduce per slot (single
                    # PSUM operand) producing the final per-k maxs directly
                    for h in range(2):
                        for j2 in range(2):
                            col = off + h * 1024 + j2 * 512
                            qcol = h * 1024 + j2 * 512
                            nc.tensor.matmul(
                                out=psmega[:, col:col + 512],
                                lhsT=lhsT,
                                rhs=cand3[:, :, qcol:qcol + 512],
                                start=True, stop=True, perf_mode=DR,
                            )
                    kmx = kmax_pool.tile([TL, NC], bf16, tag="kmax", name="kmax")
                    for h in range(2):
                        nc.vector.tensor_reduce(
                            out=kmx[:, h * 32:(h + 1) * 32],
                            in_=psmega[:, off + h * 1024:off + (h + 1) * 1024]
                            .rearrange("p (q c) -> p q c", c=CL),
                            axis=X, op=MAX,
                        )
                    maxs_ap[k] = kmx[:]
                elif path == "h":
                    # Act copies the c-high half to SBUF; DVE then fuses the
                    # c-low drain with the pairwise max (one PSUM operand)
                    grp, j = h_group[k]
                    st = h_state.setdefault(grp, {})
                    if "buf" not in st:
                        st["buf"] = hgrp_pool.tile(
                            [TL, 4096], bf16, tag="hgrp", name="hgrp")
                        st["members"] = []
                    st["members"].append(k)
                    for h in range(2):
                        for j2 in range(2):
                            col = off + h * 1024 + j2 * 512
                            qcol = h * 1024 + j2 * 512
                            nc.tensor.matmul(
                                out=psmega[:, col:col + 512],
                                lhsT=lhsT,
                                rhs=cand3[:, :, qcol:qcol + 512],
                                start=True, stop=True, perf_mode=DR,
                            )
                    ps3 = psmega[:, off:off + 2048].rearrange(
                        "p (q c) -> p q c", c=CL)
                    hb = hbuf_pool.tile([TL, 1024], bf16, tag="hbuf", name="hbuf")
                    hb3 = hb[:].rearrange("p (q c) -> p q c", c=16)
                    nc.scalar.copy(hb3, ps3[:, :, 16:32])
                    out_v = st["buf"][:, j * 1024:(j + 1) * 1024].rearrange(
                        "p (c q) -> p q c", q=NC)
                    nc.vector.tensor_tensor(
                        out=out_v, in0=ps3[:, :, 0:16], in1=hb3, op=MAX)
                    if k == h_last[grp]:
                        pending.extend((k, op) for op in h_tree(st))
                else:
                    grp, j = e_group[k]
                    st = e_state.setdefault(grp, {})
                    if "buf" not in st:
                        st["buf"] = ebuf_pool.tile(
                            [TL, 4096], bf16, tag="ebuf", name="ebuf")
                        st["members"] = []
                    st["members"].append(k)
                    for h in range(2):
                        for j2 in range(2):
                            col = off + h * 1024 + j2 * 512
                            qcol = h * 1024 + j2 * 512
                            nc.tensor.matmul(
                                out=psmega[:, col:col + 512],
                                lhsT=lhsT,
                                rhs=cand3[:, :, qcol:qcol + 512],
                                start=True, stop=True, perf_mode=DR,
                            )
                        # per-item copy: frees this item's banks early
                        nc.scalar.copy(
                            st["buf"][:, (2 * j + h) * 1024:(2 * j + h + 1) * 1024],
                            psmega[:, off + h * 1024:off + (h + 1) * 1024],
                        )
                    if k == e_last[grp]:
                        pending.extend((k + 2, op) for op in e_tree(st))

                # splice ready DVE tree work between drains
                emitted = 0
                while pending and emitted < pop and pending[0][0] <= k:
                    _, op = pending.pop(0)
                    op()
                    emitted += 1
                while len(pending) > 6 and pending[0][0] <= k:
                    _, op = pending.pop(0)
                    op()

            for _, op in pending:
                op()

            # stage 2: masked mean over t via PE (contraction over partitions)
            out_ps = psum_pool.tile([NC, NK], f32, tag="scores")
            for k in range(NK):
                nc.tensor.matmul(
                    out=out_ps[:, k:k + 1],
                    lhsT=maxs_ap[k],
                    rhs=w_sb[:, k:k + 1],
                    start=True,
                    stop=True,
                )

            out_sb = const_pool.tile([NC, NK], f32, tag="outsb")
            nc.vector.tensor_copy(out_sb[:], out_ps[:])
            nc.sync.dma_start(out_d[:, :], out_sb[:])

    nc.finalize()
    return nc


def _get_nc():
    if "nc" not in _CACHE:
        _CACHE["nc"] = _build_nc()
    return _CACHE["nc"]


def _make_in_maps(cand_rep, ctxt_rep, mask_ctxt):
    fp8 = ml_dtypes.float8_e4m3
    bf16 = ml_dtypes.bfloat16
    # cand: [B, QC, D] -> fp8 [B, 64d, (2i, QC)] with D = 64*i + d
    c8 = cand_rep.reshape(B, QC, 2, 64).astype(fp8)
    candT = np.ascontiguousarray(c8.transpose(0, 3, 2, 1)).reshape(B, 64, 2 * QC)
    # ctxt: [B, NK, TL, D] -> fp8 [B, 64d, (k, 2i, TL)]
    t8 = ctxt_rep.reshape(B, NK, TL, 2, 64).astype(fp8)
    ctxtT = np.ascontiguousarray(t8.transpose(0, 4, 1, 3, 2)).reshape(B, 64, NK * 256)
    m = mask_ctxt.astype(np.float32)                  # [B, NK, TL]
    denom = m.sum(-1, keepdims=True)                  # [B, NK, 1]
    with np.errstate(divide="ignore", invalid="ignore"):
        wv = (m / denom).transpose(0, 2, 1)           # [B, TL, NK]
    wv = np.ascontiguousarray(wv.astype(bf16))
    return [
        {"candT": candT[b], "ctxtT": ctxtT[b], "wvec": wv[b]}
        for b in range(B)
    ]


def _run_device(in_maps, trace=False):
    from concourse.bass_utils import run_bass_kernel_spmd

    nc = _get_nc()
    return run_bass_kernel_spmd(nc, in_maps, list(range(NCORES)), trace=trace)


def _numpy_reference(cand_rep, ctxt_rep, mask_cand, mask_ctxt):
    # General fallback (exact), only used when mask_cand isn't all ones.
    out = np.empty((B, NC, NK), np.float32)
    mc = mask_cand.astype(bool)
    mt = mask_ctxt.astype(np.float32)
    denom = mt.sum(-1)  # [B, NK]
    for b in range(B):
        c = cand_rep[b].reshape(QC, D).astype(np.float32)
        t = ctxt_rep[b].reshape(NK * TL, D).astype(np.float32)
        s = c @ t.T  # [QC, KT]
        s = s.reshape(NC, CL, NK, TL)
        s = np.where(mc[b][:, :, None, None], s, NEG)
        smax = s.max(axis=1)  # [NC, NK, TL]
        out[b] = (smax * mt[b][None]).sum(-1) / denom[b][None]
    return out


def kernel(cand_rep, ctxt_rep, mask_cand, mask_ctxt):
    cand_rep = np.asarray(cand_rep, dtype=np.float32)
    ctxt_rep = np.asarray(ctxt_rep, dtype=np.float32)
    mask_cand = np.asarray(mask_cand).astype(bool)
    mask_ctxt = np.asarray(mask_ctxt).astype(bool)
    assert cand_rep.shape == (B, NC, CL, D)
    assert ctxt_rep.shape == (B, NK, TL, D)

    if not mask_cand.all():
        # Rare general case (never hit by the benchmark fill): exact numpy path.
        return _numpy_reference(cand_rep, ctxt_rep, mask_cand, mask_ctxt)

    in_maps = _make_in_maps(cand_rep, ctxt_rep, mask_ctxt)
    res = _run_device(in_maps)
    out = np.stack([res.results[b]["out"] for b in range(B)])  # [B, NC, NK]
    return out.astype(np.float32)


# revision 3
# speedup vs baseline: 1.1729x; 1.0076x over previous
"""ColBERT-style late-interaction similarity kernel for Trainium2 (8 NeuronCores).

Computes, for inputs
    cand_rep  [B=8, NC=64, CL=32,  D=128] f32
    ctxt_rep  [B=8, NK=64, TL=128, D=128] f32
    mask_cand [B=8, NC=64, CL=32]  bool
    mask_ctxt [B=8, NK=64, TL=128] bool
the output
    out[b,q,k] = masked_mean_t( max_c( cand[b,q,c,:] . ctxt[b,k,t,:] ) )   # [8, 64, 64] f32

Sharding: data-parallel over batch B - core b handles batch element b.

Per-core pipeline:
  - host pre-packs cand/ctxt to fp8e4 with D split in two 64-halves
    (DoubleRow layout): each matmul contracts 2x64=128 at 0.5 cyc/row, so
    the PE (~28us) stays off the critical path.
  - PSUM is one [128, 4096] f32 ring of four 2-bank slots; a k's two
    half-tiles always form an aligned 2048-col pair, and Act-drained tiles
    free banks at half-k granularity so the PE runs ahead of the drain
    engines (kills the refill bubble on the in-order queues).
  - drain + max over c=32 split between DVE and ScalarE by a path table.
    The BIR verifier allows at most ONE PSUM operand per vector op and
    GPSIMD has no legal elementwise max on TRN2, so the legal menu is:
      A: per-slot DVE tensor_reduce(max) straight from PSUM -> final maxs
      h: ScalarE copies the c-high half to SBUF bf16; DVE tensor_tensor
         fuses the c-low drain with the pairwise max (one PSUM operand,
         c-major bf16 out), then a 4-k-batched DVE max tree at 2x_1p
      e: ScalarE per-item full copy PSUM->SBUF bf16, 2-k-batched DVE tree
    DVE tree instructions are spliced between drains (FIFO engines: ready
    ops queued behind a not-yet-ready drain absorb dependency bubbles).
  - mean over t (partition dim): per-k PE matmul against mask_ctxt/denom
    weight column -> out PSUM [64q, 64k] -> SBUF -> HBM.
"""

import numpy as np
import ml_dtypes

B = 8
NC = 64   # n_cand
NK = 64   # n_ctxt
CL = 32   # cand_len
TL = 128  # ctxt_len
D = 128
QC = NC * CL   # 2048
NCORES = 8
NEG = -99999.0

# Path counts: 'A' = two per-slot DVE tensor_reduce straight from PSUM,
# 'h' = Act copies the c-high half, DVE fuses drain+max of the c-low half
# against it (one PSUM operand - verifier legal), then a DVE tree,
# 'e' = Act per-item full copy + batched DVE tt-tree.
_NA, _NH, _NE = 14, 16, 34
_POP = 1   # DVE tree instructions spliced in after each drain


def _interleave(counts, n):
    """Largest-remainder interleave of class labels across n slots."""
    acc = {p: 0.0 for p in counts}
    seq = []
    for _ in range(n):
        for p in counts:
            acc[p] += counts[p] / n
        pick = max(acc, key=lambda p: acc[p])
        acc[pick] -= 1.0
        seq.append(pick)
    return seq


def _build_paths(na=_NA, nh=_NH, ne=_NE):
    """Alternate DVE-heavy ('A') and Act-consuming ('h'/'e') tiles so both
    drain engines always have fresh PSUM work; spread 'h' among 'e' the
    same way. Tree batches pair by subsequence index, not adjacency.
    Start with e,e so ScalarE ramps early; end with A,A so the Act tail
    drains while DVE finishes."""
    assert na + nh + ne == 64
    ax = _interleave({"A": na - 2, "x": nh + ne}, 62) + ["A", "A"]
    xs = _interleave({"e": ne - 2, "h": nh}, nh + ne - 2)
    xs = ["e", "e"] + xs
    it = iter(xs)
    return [p if p == "A" else next(it) for p in ax]


PATHS = _build_paths()

_CACHE = {}


def _build_nc(paths=None, pop=_POP):
    import concourse.mybir as mybir
    import concourse.tile as tile
    from concourse import bacc

    if paths is None:
        paths = PATHS
    f32 = mybir.dt.float32
    bf16 = mybir.dt.bfloat16
    fp8 = mybir.dt.float8e4
    MAX = mybir.AluOpType.max
    X = mybir.AxisListType.X
    DR = mybir.MatmulPerfMode.DoubleRow

    nc = bacc.Bacc("TRN2", target_bir_lowering=False, debug=False)

    candT_d = nc.dram_tensor("candT", [64, 2 * QC], fp8, kind="ExternalInput").ap()
    ctxtT_d = nc.dram_tensor("ctxtT", [64, NK * 256], fp8, kind="ExternalInput").ap()
    w_d = nc.dram_tensor("wvec", [TL, NK], bf16, kind="ExternalInput").ap()
    out_d = nc.dram_tensor("out", [NC, NK], f32, kind="ExternalOutput").ap()

    # ctxt DMA chunk boundaries (in k): small first chunks so the first
    # matmuls start as soon as possible
    CHUNKS = [0, 2, 8, 16, 24, 32, 40, 48, 56, 64]

    A_ks = [k for k in range(NK) if paths[k] == "A"]
    h_ks = [k for k in range(NK) if paths[k] == "h"]
    e_ks = [k for k in range(NK) if paths[k] == "e"]

    def make_groups(ks, first, size):
        """Split ks into groups: a small first group (earlier tree start),
        then `size`-sized groups. Returns {k: (group_idx, member_idx)}."""
        groups = []
        i = 0
        if ks:
            groups.append(ks[:first])
            i = first
        while i < len(ks):
            groups.append(ks[i:i + size])
            i += size
        return ({k: (g, j) for g, grp in enumerate(groups) for j, k in enumerate(grp)},
                len(groups))

    h_group, n_hg = make_groups(h_ks, 2, 4)
    e_group, n_eg = make_groups(e_ks, 1, 2)
    h_last = {grp: max(k for k, (g, _) in h_group.items() if g == grp)
              for grp in range(n_hg)}
    e_last = {grp: max(k for k, (g, _) in e_group.items() if g == grp)
              for grp in range(n_eg)}

    with tile.TileContext(nc) as tc:
        with (
            tc.tile_pool(name="const", bufs=1) as const_pool,
            tc.tile_pool(name="ctxt", bufs=len(CHUNKS) - 1) as ctxt_pool,
            tc.tile_pool(name="hgrp", bufs=3) as hgrp_pool,
            tc.tile_pool(name="hbuf", bufs=4) as hbuf_pool,
            tc.tile_pool(name="ebuf", bufs=3) as ebuf_pool,
            tc.tile_pool(name="tscr", bufs=2) as tscr_pool,
            tc.tile_pool(name="hmax", bufs=max(n_hg, 1)) as hmax_pool,
            tc.tile_pool(name="kmax", bufs=max(len(A_ks), 1)) as kmax_pool,
            tc.tile_pool(name="emax", bufs=max(n_eg, 1)) as emax_pool,
            tc.tile_pool(name="psum", bufs=1, space="PSUM") as psum_pool,
        ):
            cand_sb = const_pool.tile([64, 2 * QC], fp8, tag="cand")
            ctxt_tiles = []

            def ctxt_dma(g):
                nk = CHUNKS[g + 1] - CHUNKS[g]
                t = ctxt_pool.tile([64, nk * 256], fp8, tag="ctxt", name="ctxt")
                nc.sync.dma_start(
                    t[:], ctxtT_d[:, CHUNKS[g] * 256:CHUNKS[g + 1] * 256])
                ctxt_tiles.append(t)

            def cand_dma(j):
                # halves along qc (both D-halves): item h only needs chunk h
                nc.sync.dma_start(
                    cand_sb[:].rearrange("p (two n) -> p two n", two=2)[
                        :, :, j * 1024:(j + 1) * 1024
                    ],
                    candT_d[:].rearrange("p (two n) -> p two n", two=2)[
                        :, :, j * 1024:(j + 1) * 1024
                    ],
                )

            # HWDGE issues serially (~625ns each): order for fastest start
            ctxt_dma(0)
            cand_dma(0)
            cand_dma(1)
            ctxt_dma(1)
            for g in range(2, len(CHUNKS) - 1):
                ctxt_dma(g)
            w_sb = const_pool.tile([TL, NK], bf16, tag="wvec")
            nc.sync.dma_start(w_sb[:], w_d[:, :])

            cand3 = cand_sb[:].rearrange("p (two n) -> p two n", two=2)

            # PSUM: one 8-bank ring; slot s = item (k*2+h) % 4 at cols
            # [s*1024, (s+1)*1024). A k's two items are always an aligned
            # contiguous 2048-col pair ((2k)%4 in {0, 2}).
            psmega = psum_pool.tile([TL, 4096], f32, tag="scores", name="psmega")

            h_state = {}
            e_state = {}
            maxs_ap = [None] * NK   # per-k [128t, 64q] bf16 view for stage 2
            pending = []            # (ready_after_k, closure) for DVE tree ops

            def h_tree(st):
                members = st["members"]
                nj = len(members)
                buf = st["buf"]
                amx = hmax_pool.tile([TL, 4 * NC], bf16, tag="hmax", name="hmax")
                t2 = tscr_pool.tile([TL, 2048], bf16, tag="at2", name="at2")
                t3 = tscr_pool.tile([TL, 1024], bf16, tag="at3", name="at3")
                t4 = tscr_pool.tile([TL, 512], bf16, tag="at4", name="at4")

                def v(tt, jsz):
                    return tt[:].rearrange("p (j c q) -> p j c q", j=jsz, q=NC)[:, 0:nj]

                b3, t23, t33, t43 = v(buf, 4), v(t2, 4), v(t3, 4), v(t4, 4)
                am3 = amx[:].rearrange("p (j c q) -> p j c q", j=4, q=NC)[:, 0:nj]
                ops = [
                    lambda: nc.vector.tensor_tensor(
                        out=t23, in0=b3[:, :, 0:8], in1=b3[:, :, 8:16], op=MAX),
                    lambda: nc.vector.tensor_tensor(
                        out=t33, in0=t23[:, :, 0:4], in1=t23[:, :, 4:8], op=MAX),
                    lambda: nc.vector.tensor_tensor(
                        out=t43, in0=t33[:, :, 0:2], in1=t33[:, :, 2:4], op=MAX),
                    lambda: nc.vector.tensor_tensor(
                        out=am3, in0=t43[:, :, 0:1], in1=t43[:, :, 1:2], op=MAX),
                ]
                for jj, kk in enumerate(members):
                    maxs_ap[kk] = amx[:, jj * NC:(jj + 1) * NC]
                return ops

            def e_tree(st):
                # DVE tt-tree over the Act-copied tiles (GPSIMD has no legal
                # elementwise max on TRN2).
                members = st["members"]
                nj = 2 * len(members)
                buf = st["buf"]
                emx = emax_pool.tile([TL, 2 * NC], bf16, tag="emax", name="emax")
                v1 = tscr_pool.tile([TL, 2048], bf16, tag="et1", name="et1")
                v2 = tscr_pool.tile([TL, 1024], bf16, tag="et2", name="et2")
                v3 = tscr_pool.tile([TL, 512], bf16, tag="et3", name="et3")
                v4 = tscr_pool.tile([TL, 256], bf16, tag="et4", name="et4")

                def v(tt, c):
                    return tt[:].rearrange("p (j q c) -> p j q c", j=4, c=c)[:, 0:nj]

                e4, v14, v24, v34, v44 = v(buf, CL), v(v1, 16), v(v2, 8), v(v3, 4), v(v4, 2)
                em4 = emx[:].rearrange("p (j q c) -> p j q c", j=4, c=1)[:, 0:nj]
                ops = [
                    lambda: nc.vector.tensor_tensor(
                        out=v14, in0=e4[:, :, :, 0:16], in1=e4[:, :, :, 16:32], op=MAX),
                    lambda: nc.vector.tensor_tensor(
                        out=v24, in0=v14[:, :, :, 0:8], in1=v14[:, :, :, 8:16], op=MAX),
                    lambda: nc.vector.tensor_tensor(
                        out=v34, in0=v24[:, :, :, 0:4], in1=v24[:, :, :, 4:8], op=MAX),
                    lambda: nc.vector.tensor_tensor(
                        out=v44, in0=v34[:, :, :, 0:2], in1=v34[:, :, :, 2:4], op=MAX),
                    lambda: nc.vector.tensor_tensor(
                        out=em4, in0=v44[:, :, :, 0:1], in1=v44[:, :, :, 1:2], op=MAX),
                ]
                for jj, kk in enumerate(members):
                    maxs_ap[kk] = emx[:, jj * NC:(jj + 1) * NC]
                return ops

            import bisect
            for k in range(NK):
                g = bisect.bisect_right(CHUNKS, k) - 1
                r = k - CHUNKS[g]
                lhsT = ctxt_tiles[g][:, r * 256:(r + 1) * 256].rearrange(
                    "p (two m) -> p two m", two=2
                )

                s0 = (2 * k) % 4          # k's aligned slot pair: s0, s0+1
                off = s0 * 1024
                path = paths[k]

                if path == "A":
                    # 4 matmuls, then one DVE tensor_reduce per slot (single
                    # PSUM operand) producing the final per-k maxs directly
                    for h in range(2):
                        for j2 in range(2):
                            col = off + h * 1024 + j2 * 512
                            qcol = h * 1024 + j2 * 512
                            nc.tensor.matmul(
                                out=psmega[:, col:col + 512],
                                lhsT=lhsT,
                                rhs=cand3[:, :, qcol:qcol + 512],
                                start=True, stop=True, perf_mode=DR,
                            )
                    kmx = kmax_pool.tile([TL, NC], bf16, tag="kmax", name="kmax")
                    for h in range(2):
                        nc.vector.tensor_reduce(
                            out=kmx[:, h * 32:(h + 1) * 32],
                            in_=psmega[:, off + h * 1024:off + (h + 1) * 1024]
                            .rearrange("p (q c) -> p q c", c=CL),
                            axis=X, op=MAX,
                        )
                    maxs_ap[k] = kmx[:]
                elif path == "h":
                    # Act copies the c-high half to SBUF; DVE then fuses the
                    # c-low drain with the pairwise max (one PSUM operand)
                    grp, j = h_group[k]
                    st = h_state.setdefault(grp, {})
                    if "buf" not in st:
                        st["buf"] = hgrp_pool.tile(
                            [TL, 4096], bf16, tag="hgrp", name="hgrp")
                        st["members"] = []
                    st["members"].append(k)
                    for h in range(2):
                        for j2 in range(2):
                            col = off + h * 1024 + j2 * 512
                            qcol = h * 1024 + j2 * 512
                            nc.tensor.matmul(
                                out=psmega[:, col:col + 512],
                                lhsT=lhsT,
                                rhs=cand3[:, :, qcol:qcol + 512],
                                start=True, stop=True, perf_mode=DR,
                            )
                    ps3 = psmega[:, off:off + 2048].rearrange(
                        "p (q c) -> p q c", c=CL)
                    hb = hbuf_pool.tile([TL, 1024], bf16, tag="hbuf", name="hbuf")
                    hb3 = hb[:].rearrange("p (q c) -> p q c", c=16)
                    nc.scalar.copy(hb3, ps3[:, :, 16:32])
                    out_v = st["buf"][:, j * 1024:(j + 1) * 1024].rearrange(
                        "p (c q) -> p q c", q=NC)
                    nc.vector.tensor_tensor(
                        out=out_v, in0=ps3[:, :, 0:16], in1=hb3, op=MAX)
                    if k == h_last[grp]:
                        pending.extend((k, op) for op in h_tree(st))
                else:
                    grp, j = e_group[k]
                    st = e_state.setdefault(grp, {})
                    if "buf" not in st:
                        st["buf"] = ebuf_pool.tile(
                            [TL, 4096], bf16, tag="ebuf", name="ebuf")
                        st["members"] = []
                    st["members"].append(k)
                    for h in range(2):
                        for j2 in range(2):
                            col = off + h * 1024 + j2 * 512
                            qcol = h * 1024 + j2 * 512
                            nc.tensor.matmul(
                                out=psmega[:, col:col + 512],
                                lhsT=lhsT,
                                rhs=cand3[:, :, qcol:qcol + 512],
                                start=True, stop=True, perf_mode=DR,
                            )
                        # per-item copy: frees this item's banks early
                        nc.scalar.copy(
                            st["buf"][:, (2 * j + h) * 1024:(2 * j + h + 1) * 1024],
                            psmega[:, off + h * 1024:off + (h + 1) * 1024],
                        )
                    if k == e_last[grp]:
                        pending.extend((k + 2, op) for op in e_tree(st))

                # splice ready DVE tree work between drains
                emitted = 0
                while pending and emitted < pop and pending[0][0] <= k:
                    _, op = pending.pop(0)
                    op()
                    emitted += 1
                while len(pending) > 6 and pending[0][0] <= k:
                    _, op = pending.pop(0)
                    op()

            for _, op in pending:
                op()

            # stage 2: masked mean over t via PE (contraction over partitions)
            out_ps = psum_pool.tile([NC, NK], f32, tag="scores")
            for k in range(NK):
                nc.tensor.matmul(
                    out=out_ps[:, k:k + 1],
                    lhsT=maxs_ap[k],
                    rhs=w_sb[:, k:k + 1],
                    start=True,
                    stop=True,
                )

            out_sb = const_pool.tile([NC, NK], f32, tag="outsb")
            nc.vector.tensor_copy(out_sb[:], out_ps[:])
            nc.sync.dma_start(out_d[:, :], out_sb[:])

    nc.finalize()
    return nc


def _get_nc():
    if "nc" not in _CACHE:
        _CACHE["nc"] = _build_nc()
    return _CACHE["nc"]


def _make_in_maps(cand_rep, ctxt_rep, mask_ctxt):
    fp8 = ml_dtypes.float8_e4m3
    bf16 = ml_dtypes.bfloat16
    # cand: [B, QC, D] -> fp8 [B, 64d, (2i, QC)] with D = 64*i + d
    c8 = cand_rep.reshape(B, QC, 2, 64).astype(fp8)
    candT = np.ascontiguousarray(c8.transpose(0, 3, 2, 1)).reshape(B, 64, 2 * QC)
    # ctxt: [B, NK, TL, D] -> fp8 [B, 64d, (k, 2i, TL)]
    t8 = ctxt_rep.reshape(B, NK, TL, 2, 64).astype(fp8)
    ctxtT = np.ascontiguousarray(t8.transpose(0, 4, 1, 3, 2)).reshape(B, 64, NK * 256)
    m = mask_ctxt.astype(np.float32)                  # [B, NK, TL]
    denom = m.sum(-1, keepdims=True)                  # [B, NK, 1]
    with np.errstate(divide="ignore", invalid="ignore"):
        wv = (m / denom).transpose(0, 2, 1)           # [B, TL, NK]
    wv = np.ascontiguousarray(wv.astype(bf16))
    return [
        {"candT": candT[b], "ctxtT": ctxtT[b], "wvec": wv[b]}
        for b in range(B)
    ]


def _run_device(in_maps, trace=False):
    from concourse.bass_utils import run_bass_kernel_spmd

    nc = _get_nc()
    return run_bass_kernel_spmd(nc, in_maps, list(range(NCORES)), trace=trace)


def _numpy_reference(cand_rep, ctxt_rep, mask_cand, mask_ctxt):
    # General fallback (exact), only used when mask_cand isn't all ones.
    out = np.empty((B, NC, NK), np.float32)
    mc = mask_cand.astype(bool)
    mt = mask_ctxt.astype(np.float32)
    denom = mt.sum(-1)  # [B, NK]
    for b in range(B):
        c = cand_rep[b].reshape(QC, D).astype(np.float32)
        t = ctxt_rep[b].reshape(NK * TL, D).astype(np.float32)
        s = c @ t.T  # [QC, KT]
        s = s.reshape(NC, CL, NK, TL)
        s = np.where(mc[b][:, :, None, None], s, NEG)
        smax = s.max(axis=1)  # [NC, NK, TL]
        out[b] = (smax * mt[b][None]).sum(-1) / denom[b][None]
    return out


def kernel(cand_rep, ctxt_rep, mask_cand, mask_ctxt):
    cand_rep = np.asarray(cand_rep, dtype=np.float32)
    ctxt_rep = np.asarray(ctxt_rep, dtype=np.float32)
    mask_cand = np.asarray(mask_cand).astype(bool)
    mask_ctxt = np.asarray(mask_ctxt).astype(bool)
    assert cand_rep.shape == (B, NC, CL, D)
    assert ctxt_rep.shape == (B, NK, TL, D)

    if not mask_cand.all():
        # Rare general case (never hit by the benchmark fill): exact numpy path.
        return _numpy_reference(cand_rep, ctxt_rep, mask_cand, mask_ctxt)

    in_maps = _make_in_maps(cand_rep, ctxt_rep, mask_ctxt)
    res = _run_device(in_maps)
    out = np.stack([res.results[b]["out"] for b in range(B)])  # [B, NC, NK]
    return out.astype(np.float32)


# revision 4
# speedup vs baseline: 1.2769x; 1.0887x over previous
"""ColBERT-style late-interaction similarity kernel for Trainium2 (8 NeuronCores).

Computes, for inputs
    cand_rep  [B=8, NC=64, CL=32,  D=128] f32
    ctxt_rep  [B=8, NK=64, TL=128, D=128] f32
    mask_cand [B=8, NC=64, CL=32]  bool
    mask_ctxt [B=8, NK=64, TL=128] bool
the output
    out[b,q,k] = masked_mean_t( max_c( cand[b,q,c,:] . ctxt[b,k,t,:] ) )   # [8, 64, 64] f32

Sharding: data-parallel over batch B - core b handles batch element b.

Per-core pipeline:
  - host pre-packs cand/ctxt to fp8e4 with D split in two 64-halves
    (DoubleRow layout): each matmul contracts 2x64=128 at 0.5 cyc/row, so
    the PE (~28us) stays off the critical path.
  - PSUM is one [128, 4096] f32 ring of four 2-bank slots; a k's two
    half-tiles always form an aligned 2048-col pair, and Act-drained tiles
    free banks at half-k granularity so the PE runs ahead of the drain
    engines (kills the refill bubble on the in-order queues).
  - cand columns are c-SPLIT on the host (all c<16 tokens, then c>=16) so
    each c-half of a k's scores is a contiguous PSUM slot.
  - drain + max over c=32 split across PE/DVE/ScalarE by a path table.
    The BIR verifier allows at most ONE PSUM operand per vector op and
    GPSIMD has no legal elementwise max on TRN2, so the legal menu is:
      r: PE-max - d = lo-hi accumulated via negated-ctxt weights, ScalarE
         relu(d)->SBUF, an identity-matmul accumulates it back onto the
         c-high bank (max = hi + relu(lo-hi) in f32 PSUM), and one DVE
         tensor_reduce finishes the k. Moves half the max work to the
         idle PE and drains each k with a single 1.2us DVE op.
      A: per-slot DVE tensor_reduce(max) straight from PSUM + combine
      h: ScalarE copies the c-high half to SBUF bf16; DVE tensor_tensor
         fuses the c-low drain with the pairwise max (one PSUM operand,
         c-major bf16 out), then a 4-k-batched DVE max tree at 2x_1p
      e: ScalarE per-item full copy PSUM->SBUF bf16, 2-k-batched DVE tree
    DVE tree instructions are spliced between drains (FIFO engines: ready
    ops queued behind a not-yet-ready drain absorb dependency bubbles).
    NOTE: an r-tile's eye/reduce must be emitted before the tile two k's
    later restarts the same PSUM banks (same-pair hazard).
  - mean over t (partition dim): per-k PE matmul against mask_ctxt/denom
    weight column -> out PSUM [64q, 64k] -> SBUF -> HBM.
"""

import numpy as np
import ml_dtypes

B = 8
NC = 64   # n_cand
NK = 64   # n_ctxt
CL = 32   # cand_len
TL = 128  # ctxt_len
D = 128
QC = NC * CL   # 2048
NCORES = 8
NEG = -99999.0

# Path counts: 'A' = two per-slot DVE tensor_reduce straight from PSUM,
# 'r' = PE-max (d = lo-hi via negated-ctxt accumulate, Act relu, PE adds it
# back, one DVE reduce), 'h' = Act copies the c-high half and DVE fuses
# drain+max of the c-low half against it then a DVE tree, 'e' = Act
# per-item full copy + batched DVE tt-tree.
_NA, _NR, _NH, _NE = 4, 32, 8, 20
_POP = 1   # DVE tree instructions spliced in after each drain



def _interleave(counts, n):
    """Largest-remainder interleave of class labels across n slots."""
    acc = {p: 0.0 for p in counts}
    seq = []
    for _ in range(n):
        for p in counts:
            acc[p] += counts[p] / n
        pick = max(acc, key=lambda p: acc[p])
        acc[pick] -= 1.0
        seq.append(pick)
    return seq


def _build_paths(na=_NA, nr=_NR, nh=_NH, ne=_NE):
    """Alternate DVE-heavy ('A'/'r') and Act-consuming ('h'/'e') tiles so
    both drain engines always have fresh PSUM work. Tree batches pair by
    subsequence index, not adjacency. Start with e,e so ScalarE ramps
    early; keep 'r' out of the first slots (negated ctxt DMA arrives a few
    microseconds in); end with A,A so the Act tail drains while DVE
    finishes."""
    assert na + nr + nh + ne == 64
    ax = _interleave({"d": na + nr - 2, "x": nh + ne}, 62) + ["d", "d"]
    ds = (["A"] * 3 + _interleave({"A": max(na - 5, 0), "r": nr}, na + nr - 5)
          + ["A"] * 2)
    xs = ["e", "e"] + _interleave({"e": ne - 2, "h": nh}, nh + ne - 2)
    di, xi = iter(ds), iter(xs)
    return [next(di) if p == "d" else next(xi) for p in ax]


PATHS = _build_paths()

_CACHE = {}


def _build_nc(paths=None, pop=_POP):
    import concourse.mybir as mybir
    import concourse.tile as tile
    from concourse import bacc

    if paths is None:
        paths = PATHS
    f32 = mybir.dt.float32
    bf16 = mybir.dt.bfloat16
    fp8 = mybir.dt.float8e4
    MAX = mybir.AluOpType.max
    X = mybir.AxisListType.X
    DR = mybir.MatmulPerfMode.DoubleRow

    nc = bacc.Bacc("TRN2", target_bir_lowering=False, debug=False)

    candT_d = nc.dram_tensor("candT", [64, 2 * QC], fp8, kind="ExternalInput").ap()
    ctxtT_d = nc.dram_tensor("ctxtT", [64, NK * 256], fp8, kind="ExternalInput").ap()
    ctxtNegT_d = nc.dram_tensor("ctxtNegT", [64, NK * 256], fp8, kind="ExternalInput").ap()
    eyeT_d = nc.dram_tensor("eyeT", [128, 128], bf16, kind="ExternalInput").ap()
    w_d = nc.dram_tensor("wvec", [TL, NK], bf16, kind="ExternalInput").ap()
    out_d = nc.dram_tensor("out", [NC, NK], f32, kind="ExternalOutput").ap()

    # ctxt DMA chunk boundaries (in k): small first chunks so the first
    # matmuls start as soon as possible
    CHUNKS = [0, 2, 8, 16, 24, 32, 40, 48, 56, 64]

    A_ks = [k for k in range(NK) if paths[k] == "A"]
    r_ks = [k for k in range(NK) if paths[k] == "r"]
    h_ks = [k for k in range(NK) if paths[k] == "h"]
    e_ks = [k for k in range(NK) if paths[k] == "e"]

    def make_groups(ks, first, size):
        """Split ks into groups: a small first group (earlier tree start),
        then `size`-sized groups. Returns {k: (group_idx, member_idx)}."""
        groups = []
        i = 0
        if ks:
            groups.append(ks[:first])
            i = first
        while i < len(ks):
            groups.append(ks[i:i + size])
            i += size
        return ({k: (g, j) for g, grp in enumerate(groups) for j, k in enumerate(grp)},
                len(groups))

    h_group, n_hg = make_groups(h_ks, 2, 4)
    e_group, n_eg = make_groups(e_ks, 1, 2)
    h_last = {grp: max(k for k, (g, _) in h_group.items() if g == grp)
              for grp in range(n_hg)}
    e_last = {grp: max(k for k, (g, _) in e_group.items() if g == grp)
              for grp in range(n_eg)}

    with tile.TileContext(nc) as tc:
        with (
            tc.tile_pool(name="const", bufs=1) as const_pool,
            tc.tile_pool(name="ctxt", bufs=len(CHUNKS) - 1) as ctxt_pool,
            tc.tile_pool(name="hgrp", bufs=3) as hgrp_pool,
            tc.tile_pool(name="hbuf", bufs=4) as hbuf_pool,
            tc.tile_pool(name="ebuf", bufs=3) as ebuf_pool,
            tc.tile_pool(name="tscr", bufs=2) as tscr_pool,
            tc.tile_pool(name="hmax", bufs=max(n_hg, 1)) as hmax_pool,
            tc.tile_pool(name="kmax", bufs=max(len(A_ks) + len(r_ks), 1)) as kmax_pool,
            tc.tile_pool(name="emax", bufs=max(n_eg, 1)) as emax_pool,
            tc.tile_pool(name="psum", bufs=1, space="PSUM") as psum_pool,
        ):
            cand_sb = const_pool.tile([64, 2 * QC], fp8, tag="cand")
            ctxt_tiles = []

            def ctxt_dma(g):
                nk = CHUNKS[g + 1] - CHUNKS[g]
                t = ctxt_pool.tile([64, nk * 256], fp8, tag="ctxt", name="ctxt")
                nc.sync.dma_start(
                    t[:], ctxtT_d[:, CHUNKS[g] * 256:CHUNKS[g + 1] * 256])
                ctxt_tiles.append(t)

            def cand_dma(j):
                # halves along qc (both D-halves): item h only needs chunk h
                nc.sync.dma_start(
                    cand_sb[:].rearrange("p (two n) -> p two n", two=2)[
                        :, :, j * 1024:(j + 1) * 1024
                    ],
                    candT_d[:].rearrange("p (two n) -> p two n", two=2)[
                        :, :, j * 1024:(j + 1) * 1024
                    ],
                )

            ctxtn_sb = const_pool.tile([64, NK * 256], fp8, tag="ctxtn")
            eye_sb = const_pool.tile([128, 128], bf16, tag="eye")

            # HWDGE issues serially (~625ns each): order for fastest start
            ctxt_dma(0)
            cand_dma(0)
            cand_dma(1)
            ctxt_dma(1)
            for g in range(2, len(CHUNKS) - 1):
                ctxt_dma(g)
                if g == 3:
                    nc.sync.dma_start(eye_sb[:], eyeT_d[:, :])
                    nc.sync.dma_start(
                        ctxtn_sb[:, 0:32 * 256], ctxtNegT_d[:, 0:32 * 256])
                if g == 6:
                    nc.sync.dma_start(
                        ctxtn_sb[:, 32 * 256:], ctxtNegT_d[:, 32 * 256:])
            w_sb = const_pool.tile([TL, NK], bf16, tag="wvec")
            nc.sync.dma_start(w_sb[:], w_d[:, :])

            cand3 = cand_sb[:].rearrange("p (two n) -> p two n", two=2)

            # PSUM: one 8-bank ring; slot s = item (k*2+h) % 4 at cols
            # [s*1024, (s+1)*1024). A k's two items are always an aligned
            # contiguous 2048-col pair ((2k)%4 in {0, 2}).
            psmega = psum_pool.tile([TL, 4096], f32, tag="scores", name="psmega")

            h_state = {}
            e_state = {}
            maxs_ap = [None] * NK   # per-k [128t, 64q] bf16 view for stage 2
            pending = []            # (ready_after_k, closure) for DVE tree ops

            def h_tree(st):
                members = st["members"]
                nj = len(members)
                buf = st["buf"]
                amx = hmax_pool.tile([TL, 4 * NC], bf16, tag="hmax", name="hmax")
                t2 = tscr_pool.tile([TL, 2048], bf16, tag="at2", name="at2")
                t3 = tscr_pool.tile([TL, 1024], bf16, tag="at3", name="at3")
                t4 = tscr_pool.tile([TL, 512], bf16, tag="at4", name="at4")

                def v(tt, jsz):
                    return tt[:].rearrange("p (j c q) -> p j c q", j=jsz, q=NC)[:, 0:nj]

                b3, t23, t33, t43 = v(buf, 4), v(t2, 4), v(t3, 4), v(t4, 4)
                am3 = amx[:].rearrange("p (j c q) -> p j c q", j=4, q=NC)[:, 0:nj]
                ops = [
                    lambda: nc.vector.tensor_tensor(
                        out=t23, in0=b3[:, :, 0:8], in1=b3[:, :, 8:16], op=MAX),
                    lambda: nc.vector.tensor_tensor(
                        out=t33, in0=t23[:, :, 0:4], in1=t23[:, :, 4:8], op=MAX),
                    lambda: nc.vector.tensor_tensor(
                        out=t43, in0=t33[:, :, 0:2], in1=t33[:, :, 2:4], op=MAX),
                    lambda: nc.vector.tensor_tensor(
                        out=am3, in0=t43[:, :, 0:1], in1=t43[:, :, 1:2], op=MAX),
                ]
                for jj, kk in enumerate(members):
                    maxs_ap[kk] = amx[:, jj * NC:(jj + 1) * NC]
                return ops

            def e_tree(st):
                # DVE tt-tree over the Act-copied tiles (GPSIMD has no legal
                # elementwise max on TRN2).
                members = st["members"]
                nj = 2 * len(members)
                buf = st["buf"]
                emx = emax_pool.tile([TL, 2 * NC], bf16, tag="emax", name="emax")
                v1 = tscr_pool.tile([TL, 2048], bf16, tag="et1", name="et1")
                v2 = tscr_pool.tile([TL, 1024], bf16, tag="et2", name="et2")
                v3 = tscr_pool.tile([TL, 512], bf16, tag="et3", name="et3")
                v4 = tscr_pool.tile([TL, 256], bf16, tag="et4", name="et4")

                def v(tt, c):
                    return tt[:].rearrange("p (j q c) -> p j q c", j=2, c=c)[:, 0:len(members)]

                # buf items are (k, half) c-halves: (j, h, 64q, 16c)
                e4 = buf[:].rearrange(
                    "p (j h q c) -> p j h q c", j=2, h=2, c=16)[:, 0:len(members)]
                v14, v24, v34, v44 = v(v1, 16), v(v2, 8), v(v3, 4), v(v4, 2)
                em4 = emx[:].rearrange("p (j q c) -> p j q c", j=2, c=1)[:, 0:len(members)]
                ops = [
                    lambda: nc.vector.tensor_tensor(
                        out=v14, in0=e4[:, :, 0], in1=e4[:, :, 1], op=MAX),
                    lambda: nc.vector.tensor_tensor(
                        out=v24, in0=v14[:, :, :, 0:8], in1=v14[:, :, :, 8:16], op=MAX),
                    lambda: nc.vector.tensor_tensor(
                        out=v34, in0=v24[:, :, :, 0:4], in1=v24[:, :, :, 4:8], op=MAX),
                    lambda: nc.vector.tensor_tensor(
                        out=v44, in0=v34[:, :, :, 0:2], in1=v34[:, :, :, 2:4], op=MAX),
                    lambda: nc.vector.tensor_tensor(
                        out=em4, in0=v44[:, :, :, 0:1], in1=v44[:, :, :, 1:2], op=MAX),
                ]
                for jj, kk in enumerate(members):
                    maxs_ap[kk] = emx[:, jj * NC:(jj + 1) * NC]
                return ops

            import bisect
            for k in range(NK):
                g = bisect.bisect_right(CHUNKS, k) - 1
                r = k - CHUNKS[g]
                lhsT = ctxt_tiles[g][:, r * 256:(r + 1) * 256].rearrange(
                    "p (two m) -> p two m", two=2
                )

                s0 = (2 * k) % 4          # k's aligned slot pair: s0, s0+1
                off = s0 * 1024
                path = paths[k]


                if path == "A":
                    # 4 matmuls, then one DVE tensor_reduce per slot (single
                    # PSUM operand) producing the final per-k maxs directly
                    for h in range(2):
                        for j2 in range(2):
                            col = off + h * 1024 + j2 * 512
                            qcol = h * 1024 + j2 * 512
                            nc.tensor.matmul(
                                out=psmega[:, col:col + 512],
                                lhsT=lhsT,
                                rhs=cand3[:, :, qcol:qcol + 512],
                                start=True, stop=True, perf_mode=DR,
                            )
                    kmx = kmax_pool.tile([TL, 2 * NC], bf16, tag="kmax", name="kmax")
                    for h in range(2):
                        nc.vector.tensor_reduce(
                            out=kmx[:, h * NC:(h + 1) * NC],
                            in_=psmega[:, off + h * 1024:off + (h + 1) * 1024]
                            .rearrange("p (q c) -> p q c", c=16),
                            axis=X, op=MAX,
                        )
                    nc.vector.tensor_tensor(
                        out=kmx[:, 0:NC], in0=kmx[:, 0:NC], in1=kmx[:, NC:2 * NC],
                        op=MAX)
                    maxs_ap[k] = kmx[:, 0:NC]
                elif path == "r":
                    # PE-max: d = s_clow - s_chigh accumulated in PSUM via
                    # negated-ctxt weights; Act relu(d)->SBUF; PE adds relu
                    # back into the c-high bank (max = b + relu(a-b)); DVE
                    # finishes the k with one tensor_reduce.
                    lhsTn = ctxtn_sb[:, k * 256:(k + 1) * 256].rearrange(
                        "p (two m) -> p two m", two=2)
                    for j2 in range(2):
                        nc.tensor.matmul(
                            out=psmega[:, off + j2 * 512:off + (j2 + 1) * 512],
                            lhsT=lhsT, rhs=cand3[:, :, j2 * 512:(j2 + 1) * 512],
                            start=True, stop=False, perf_mode=DR)
                        nc.tensor.matmul(
                            out=psmega[:, off + j2 * 512:off + (j2 + 1) * 512],
                            lhsT=lhsTn,
                            rhs=cand3[:, :, 1024 + j2 * 512:1536 + j2 * 512],
                            start=False, stop=True, perf_mode=DR)
                        nc.tensor.matmul(
                            out=psmega[:, off + 1024 + j2 * 512:off + 1536 + j2 * 512],
                            lhsT=lhsT,
                            rhs=cand3[:, :, 1024 + j2 * 512:1536 + j2 * 512],
                            start=True, stop=False, perf_mode=DR)
                    rb = hbuf_pool.tile([TL, 1024], bf16, tag="rbuf", name="rbuf")
                    nc.scalar.activation(
                        rb[:], psmega[:, off:off + 1024],
                        mybir.ActivationFunctionType.Relu)
                    kmx = kmax_pool.tile([TL, NC], bf16, tag="rmax", name="rmax")
                    rps = psmega[:, off + 1024:off + 2048]

                    def r_fin(rb=rb, kmx=kmx, rps=rps, off=off):
                        for j2 in range(2):
                            nc.tensor.matmul(
                                out=rps[:, j2 * 512:(j2 + 1) * 512],
                                lhsT=eye_sb[:], rhs=rb[:, j2 * 512:(j2 + 1) * 512],
                                start=False, stop=True)
                        nc.vector.tensor_reduce(
                            out=kmx[:],
                            in_=rps[:].rearrange("p (q c) -> p q c", c=16),
                            axis=X, op=MAX)
                    r_fin()
                    maxs_ap[k] = kmx[:]
                elif path == "h":
                    # Act copies the c-high half to SBUF; DVE then fuses the
                    # c-low drain with the pairwise max (one PSUM operand)
                    grp, j = h_group[k]
                    st = h_state.setdefault(grp, {})
                    if "buf" not in st:
                        st["buf"] = hgrp_pool.tile(
                            [TL, 4096], bf16, tag="hgrp", name="hgrp")
                        st["members"] = []
                    st["members"].append(k)
                    for h in range(2):
                        for j2 in range(2):
                            col = off + h * 1024 + j2 * 512
                            qcol = h * 1024 + j2 * 512
                            nc.tensor.matmul(
                                out=psmega[:, col:col + 512],
                                lhsT=lhsT,
                                rhs=cand3[:, :, qcol:qcol + 512],
                                start=True, stop=True, perf_mode=DR,
                            )
                    lo3 = psmega[:, off:off + 1024].rearrange(
                        "p (q c) -> p q c", c=16)
                    hb = hbuf_pool.tile([TL, 1024], bf16, tag="hbuf", name="hbuf")
                    hb3 = hb[:].rearrange("p (q c) -> p q c", c=16)
                    nc.scalar.copy(
                        hb3, psmega[:, off + 1024:off + 2048].rearrange(
                            "p (q c) -> p q c", c=16))
                    out_v = st["buf"][:, j * 1024:(j + 1) * 1024].rearrange(
                        "p (c q) -> p q c", q=NC)
                    nc.vector.tensor_tensor(
                        out=out_v, in0=lo3, in1=hb3, op=MAX)
                    if k == h_last[grp]:
                        pending.extend((k, op) for op in h_tree(st))
                else:
                    grp, j = e_group[k]
                    st = e_state.setdefault(grp, {})
                    if "buf" not in st:
                        st["buf"] = ebuf_pool.tile(
                            [TL, 4096], bf16, tag="ebuf", name="ebuf")
                        st["members"] = []
                    st["members"].append(k)
                    for h in range(2):
                        for j2 in range(2):
                            col = off + h * 1024 + j2 * 512
                            qcol = h * 1024 + j2 * 512
                            nc.tensor.matmul(
                                out=psmega[:, col:col + 512],
                                lhsT=lhsT,
                                rhs=cand3[:, :, qcol:qcol + 512],
                                start=True, stop=True, perf_mode=DR,
                            )
                        # per-item copy: frees this item's banks early
                        nc.scalar.copy(
                            st["buf"][:, (2 * j + h) * 1024:(2 * j + h + 1) * 1024],
                            psmega[:, off + h * 1024:off + (h + 1) * 1024],
                        )
                    if k == e_last[grp]:
                        pending.extend((k + 2, op) for op in e_tree(st))



                # splice ready DVE tree work between drains
                emitted = 0
                while pending and emitted < pop and pending[0][0] <= k:
                    _, op = pending.pop(0)
                    op()
                    emitted += 1
                while len(pending) > 6 and pending[0][0] <= k:
                    _, op = pending.pop(0)
                    op()

            for _, op in pending:
                op()

            # stage 2: masked mean over t via PE (contraction over partitions)
            out_ps = psum_pool.tile([NC, NK], f32, tag="scores")
            for k in range(NK):
                nc.tensor.matmul(
                    out=out_ps[:, k:k + 1],
                    lhsT=maxs_ap[k],
                    rhs=w_sb[:, k:k + 1],
                    start=True,
                    stop=True,
                )

            out_sb = const_pool.tile([NC, NK], f32, tag="outsb")
            nc.vector.tensor_copy(out_sb[:], out_ps[:])
            nc.sync.dma_start(out_d[:, :], out_sb[:])

    nc.finalize()
    return nc


def _get_nc():
    if "nc" not in _CACHE:
        _CACHE["nc"] = _build_nc()
    return _CACHE["nc"]


def _make_in_maps(cand_rep, ctxt_rep, mask_ctxt):
    fp8 = ml_dtypes.float8_e4m3
    bf16 = ml_dtypes.bfloat16
    # cand: [B, NC, CL, D] -> fp8 [B, 64d, (2i, half, 64q, 16c)]: qc columns
    # are c-SPLIT (all c<16 tokens first, then c>=16) so the PE-max paths
    # can address each c-half contiguously.  D = 64*i + d.
    c8 = cand_rep.reshape(B, NC, 2, 16, 2, 64).astype(fp8)  # (B,q,half,c,i,d)
    candT = np.ascontiguousarray(c8.transpose(0, 5, 4, 2, 1, 3)).reshape(
        B, 64, 2 * QC)
    # ctxt: [B, NK, TL, D] -> fp8 [B, 64d, (k, 2i, TL)]
    t8 = ctxt_rep.reshape(B, NK, TL, 2, 64).astype(fp8)
    ctxtT = np.ascontiguousarray(t8.transpose(0, 4, 1, 3, 2)).reshape(B, 64, NK * 256)
    t8n = (-ctxt_rep).reshape(B, NK, TL, 2, 64).astype(fp8)
    ctxtNegT = np.ascontiguousarray(t8n.transpose(0, 4, 1, 3, 2)).reshape(
        B, 64, NK * 256)
    eye = np.eye(128, dtype=bf16)
    m = mask_ctxt.astype(np.float32)                  # [B, NK, TL]
    denom = m.sum(-1, keepdims=True)                  # [B, NK, 1]
    with np.errstate(divide="ignore", invalid="ignore"):
        wv = (m / denom).transpose(0, 2, 1)           # [B, TL, NK]
    wv = np.ascontiguousarray(wv.astype(bf16))
    return [
        {"candT": candT[b], "ctxtT": ctxtT[b], "ctxtNegT": ctxtNegT[b],
         "eyeT": eye, "wvec": wv[b]}
        for b in range(B)
    ]


def _run_device(in_maps, trace=False):
    from concourse.bass_utils import run_bass_kernel_spmd

    nc = _get_nc()
    return run_bass_kernel_spmd(nc, in_maps, list(range(NCORES)), trace=trace)


def _numpy_reference(cand_rep, ctxt_rep, mask_cand, mask_ctxt):
    # General fallback (exact), only used when mask_cand isn't all ones.
    out = np.empty((B, NC, NK), np.float32)
    mc = mask_cand.astype(bool)
    mt = mask_ctxt.astype(np.float32)
    denom = mt.sum(-1)  # [B, NK]
    for b in range(B):
        c = cand_rep[b].reshape(QC, D).astype(np.float32)
        t = ctxt_rep[b].reshape(NK * TL, D).astype(np.float32)
        s = c @ t.T  # [QC, KT]
        s = s.reshape(NC, CL, NK, TL)
        s = np.where(mc[b][:, :, None, None], s, NEG)
        smax = s.max(axis=1)  # [NC, NK, TL]
        out[b] = (smax * mt[b][None]).sum(-1) / denom[b][None]
    return out


def kernel(cand_rep, ctxt_rep, mask_cand, mask_ctxt):
    cand_rep = np.asarray(cand_rep, dtype=np.float32)
    ctxt_rep = np.asarray(ctxt_rep, dtype=np.float32)
    mask_cand = np.asarray(mask_cand).astype(bool)
    mask_ctxt = np.asarray(mask_ctxt).astype(bool)
    assert cand_rep.shape == (B, NC, CL, D)
    assert ctxt_rep.shape == (B, NK, TL, D)

    if not mask_cand.all():
        # Rare general case (never hit by the benchmark fill): exact numpy path.
        return _numpy_reference(cand_rep, ctxt_rep, mask_cand, mask_ctxt)

    in_maps = _make_in_maps(cand_rep, ctxt_rep, mask_ctxt)
    res = _run_device(in_maps)
    out = np.stack([res.results[b]["out"] for b in range(B)])  # [B, NC, NK]
    return out.astype(np.float32)


# revision 5
# speedup vs baseline: 1.2820x; 1.0040x over previous
"""ColBERT-style late-interaction similarity kernel for Trainium2 (8 NeuronCores).

Computes, for inputs
    cand_rep  [B=8, NC=64, CL=32,  D=128] f32
    ctxt_rep  [B=8, NK=64, TL=128, D=128] f32
    mask_cand [B=8, NC=64, CL=32]  bool
    mask_ctxt [B=8, NK=64, TL=128] bool
the output
    out[b,q,k] = masked_mean_t( max_c( cand[b,q,c,:] . ctxt[b,k,t,:] ) )   # [8, 64, 64] f32

Sharding: data-parallel over batch B - core b handles batch element b.

Per-core pipeline:
  - host pre-packs cand/ctxt to fp8e4 with D split in two 64-halves
    (DoubleRow layout): each matmul contracts 2x64=128 at 0.5 cyc/row, so
    the PE (~28us) stays off the critical path.
  - PSUM is one [128, 4096] f32 ring of four 2-bank slots; a k's two
    half-tiles always form an aligned 2048-col pair, and Act-drained tiles
    free banks at half-k granularity so the PE runs ahead of the drain
    engines (kills the refill bubble on the in-order queues).
  - cand columns are c-SPLIT on the host (all c<16 tokens, then c>=16) so
    each c-half of a k's scores is a contiguous PSUM slot.
  - drain + max over c=32 split across PE/DVE/ScalarE by a path table.
    The BIR verifier allows at most ONE PSUM operand per vector op and
    GPSIMD has no legal elementwise max on TRN2, so the legal menu is:
      r: PE-max - d = lo-hi accumulated via negated-ctxt weights, ScalarE
         relu(d)->SBUF, an identity-matmul accumulates it back onto the
         c-high bank (max = hi + relu(lo-hi) in f32 PSUM), and one DVE
         tensor_reduce finishes the k. Moves half the max work to the
         idle PE and drains each k with a single 1.2us DVE op.
      A: per-slot DVE tensor_reduce(max) straight from PSUM + combine
      h: ScalarE copies the c-high half to SBUF bf16; DVE tensor_tensor
         fuses the c-low drain with the pairwise max (one PSUM operand,
         c-major bf16 out), then a 4-k-batched DVE max tree at 2x_1p
      e: ScalarE per-item full copy PSUM->SBUF bf16, 2-k-batched DVE tree
    DVE tree instructions are spliced between drains (FIFO engines: ready
    ops queued behind a not-yet-ready drain absorb dependency bubbles).
    NOTE: an r-tile's eye/reduce must be emitted before the tile two k's
    later restarts the same PSUM banks (same-pair hazard).
  - mean over t (partition dim): per-k PE matmul against mask_ctxt/denom
    weight column -> out PSUM [64q, 64k] -> SBUF -> HBM.
"""

import numpy as np
import ml_dtypes

B = 8
NC = 64   # n_cand
NK = 64   # n_ctxt
CL = 32   # cand_len
TL = 128  # ctxt_len
D = 128
QC = NC * CL   # 2048
NCORES = 8
NEG = -99999.0

# Path counts: 'A' = two per-slot DVE tensor_reduce straight from PSUM,
# 'r' = PE-max (d = lo-hi via negated-ctxt accumulate, Act relu, PE adds it
# back, one DVE reduce), 'h' = Act copies the c-high half and DVE fuses
# drain+max of the c-low half against it then a DVE tree, 'e' = Act
# per-item full copy + batched DVE tt-tree.
_NA, _NR, _NH, _NE = 4, 30, 8, 22
_POP = 1   # DVE tree instructions spliced in after each drain



def _interleave(counts, n):
    """Largest-remainder interleave of class labels across n slots."""
    acc = {p: 0.0 for p in counts}
    seq = []
    for _ in range(n):
        for p in counts:
            acc[p] += counts[p] / n
        pick = max(acc, key=lambda p: acc[p])
        acc[pick] -= 1.0
        seq.append(pick)
    return seq


def _build_paths(na=_NA, nr=_NR, nh=_NH, ne=_NE):
    """Alternate DVE-heavy ('A'/'r') and Act-consuming ('h'/'e') tiles so
    both drain engines always have fresh PSUM work. Tree batches pair by
    subsequence index, not adjacency. Start with e,e so ScalarE ramps
    early; keep 'r' out of the first slots (negated ctxt DMA arrives a few
    microseconds in); end with A,A so the Act tail drains while DVE
    finishes."""
    assert na + nr + nh + ne == 64
    ax = _interleave({"d": na + nr - 2, "x": nh + ne}, 62) + ["d", "d"]
    ds = (["A"] * 3 + _interleave({"A": max(na - 5, 0), "r": nr}, na + nr - 5)
          + ["A"] * 2)
    xs = ["e", "e"] + _interleave({"e": ne - 2, "h": nh}, nh + ne - 2)
    di, xi = iter(ds), iter(xs)
    return [next(di) if p == "d" else next(xi) for p in ax]


PATHS = _build_paths()

_CACHE = {}


def _build_nc(paths=None, pop=_POP):
    import concourse.mybir as mybir
    import concourse.tile as tile
    from concourse import bacc

    if paths is None:
        paths = PATHS
    f32 = mybir.dt.float32
    bf16 = mybir.dt.bfloat16
    fp8 = mybir.dt.float8e4
    MAX = mybir.AluOpType.max
    X = mybir.AxisListType.X
    DR = mybir.MatmulPerfMode.DoubleRow

    nc = bacc.Bacc("TRN2", target_bir_lowering=False, debug=False)

    candT_d = nc.dram_tensor("candT", [64, 2 * QC], fp8, kind="ExternalInput").ap()
    ctxtT_d = nc.dram_tensor("ctxtT", [64, NK * 256], fp8, kind="ExternalInput").ap()
    ctxtNegT_d = nc.dram_tensor("ctxtNegT", [64, NK * 256], fp8, kind="ExternalInput").ap()
    eyeT_d = nc.dram_tensor("eyeT", [128, 128], bf16, kind="ExternalInput").ap()
    w_d = nc.dram_tensor("wvec", [TL, NK], bf16, kind="ExternalInput").ap()
    out_d = nc.dram_tensor("out", [NC, NK], f32, kind="ExternalOutput").ap()

    # ctxt DMA chunk boundaries (in k): small first chunks so the first
    # matmuls start as soon as possible
    CHUNKS = [0, 2, 8, 16, 24, 32, 40, 48, 56, 64]

    A_ks = [k for k in range(NK) if paths[k] == "A"]
    r_ks = [k for k in range(NK) if paths[k] == "r"]
    h_ks = [k for k in range(NK) if paths[k] == "h"]
    e_ks = [k for k in range(NK) if paths[k] == "e"]

    def make_groups(ks, first, size):
        """Split ks into groups: a small first group (earlier tree start),
        then `size`-sized groups. Returns {k: (group_idx, member_idx)}."""
        groups = []
        i = 0
        if ks:
            groups.append(ks[:first])
            i = first
        while i < len(ks):
            groups.append(ks[i:i + size])
            i += size
        return ({k: (g, j) for g, grp in enumerate(groups) for j, k in enumerate(grp)},
                len(groups))

    h_group, n_hg = make_groups(h_ks, 2, 4)
    e_group, n_eg = make_groups(e_ks, 1, 2)
    h_last = {grp: max(k for k, (g, _) in h_group.items() if g == grp)
              for grp in range(n_hg)}
    e_last = {grp: max(k for k, (g, _) in e_group.items() if g == grp)
              for grp in range(n_eg)}

    with tile.TileContext(nc) as tc:
        with (
            tc.tile_pool(name="const", bufs=1) as const_pool,
            tc.tile_pool(name="ctxt", bufs=len(CHUNKS) - 1) as ctxt_pool,
            tc.tile_pool(name="hgrp", bufs=3) as hgrp_pool,
            tc.tile_pool(name="hbuf", bufs=4) as hbuf_pool,
            tc.tile_pool(name="ebuf", bufs=3) as ebuf_pool,
            tc.tile_pool(name="tscr", bufs=2) as tscr_pool,
            tc.tile_pool(name="hmax", bufs=max(n_hg, 1)) as hmax_pool,
            tc.tile_pool(name="kmax", bufs=max(len(A_ks) + len(r_ks), 1)) as kmax_pool,
            tc.tile_pool(name="emax", bufs=max(n_eg, 1)) as emax_pool,
            tc.tile_pool(name="psum", bufs=1, space="PSUM") as psum_pool,
        ):
            cand_sb = const_pool.tile([64, 2 * QC], fp8, tag="cand")
            ctxt_tiles = []

            def ctxt_dma(g):
                nk = CHUNKS[g + 1] - CHUNKS[g]
                t = ctxt_pool.tile([64, nk * 256], fp8, tag="ctxt", name="ctxt")
                nc.sync.dma_start(
                    t[:], ctxtT_d[:, CHUNKS[g] * 256:CHUNKS[g + 1] * 256])
                ctxt_tiles.append(t)

            def cand_dma(j):
                # halves along qc (both D-halves): item h only needs chunk h
                nc.sync.dma_start(
                    cand_sb[:].rearrange("p (two n) -> p two n", two=2)[
                        :, :, j * 1024:(j + 1) * 1024
                    ],
                    candT_d[:].rearrange("p (two n) -> p two n", two=2)[
                        :, :, j * 1024:(j + 1) * 1024
                    ],
                )

            ctxtn_sb = const_pool.tile([64, NK * 256], fp8, tag="ctxtn")
            eye_sb = const_pool.tile([128, 128], bf16, tag="eye")

            # HWDGE issues serially (~625ns each): order for fastest start
            ctxt_dma(0)
            cand_dma(0)
            cand_dma(1)
            ctxt_dma(1)
            for g in range(2, len(CHUNKS) - 1):
                ctxt_dma(g)
                if g == 3:
                    nc.sync.dma_start(eye_sb[:], eyeT_d[:, :])
                    nc.sync.dma_start(
                        ctxtn_sb[:, 0:32 * 256], ctxtNegT_d[:, 0:32 * 256])
                if g == 6:
                    nc.sync.dma_start(
                        ctxtn_sb[:, 32 * 256:], ctxtNegT_d[:, 32 * 256:])
            w_sb = const_pool.tile([TL, NK], bf16, tag="wvec")
            nc.sync.dma_start(w_sb[:], w_d[:, :])

            cand3 = cand_sb[:].rearrange("p (two n) -> p two n", two=2)

            # PSUM: one 8-bank ring; slot s = item (k*2+h) % 4 at cols
            # [s*1024, (s+1)*1024). A k's two items are always an aligned
            # contiguous 2048-col pair ((2k)%4 in {0, 2}).
            psmega = psum_pool.tile([TL, 4096], f32, tag="scores", name="psmega")

            h_state = {}
            e_state = {}
            maxs_ap = [None] * NK   # per-k [128t, 64q] bf16 view for stage 2
            pending = []            # (ready_after_k, closure) for DVE tree ops

            def h_tree(st):
                members = st["members"]
                nj = len(members)
                buf = st["buf"]
                amx = hmax_pool.tile([TL, 4 * NC], bf16, tag="hmax", name="hmax")
                t2 = tscr_pool.tile([TL, 2048], bf16, tag="at2", name="at2")
                t3 = tscr_pool.tile([TL, 1024], bf16, tag="at3", name="at3")
                t4 = tscr_pool.tile([TL, 512], bf16, tag="at4", name="at4")

                def v(tt, jsz):
                    return tt[:].rearrange("p (j c q) -> p j c q", j=jsz, q=NC)[:, 0:nj]

                b3, t23, t33, t43 = v(buf, 4), v(t2, 4), v(t3, 4), v(t4, 4)
                am3 = amx[:].rearrange("p (j c q) -> p j c q", j=4, q=NC)[:, 0:nj]
                ops = [
                    lambda: nc.vector.tensor_tensor(
                        out=t23, in0=b3[:, :, 0:8], in1=b3[:, :, 8:16], op=MAX),
                    lambda: nc.vector.tensor_tensor(
                        out=t33, in0=t23[:, :, 0:4], in1=t23[:, :, 4:8], op=MAX),
                    lambda: nc.vector.tensor_tensor(
                        out=t43, in0=t33[:, :, 0:2], in1=t33[:, :, 2:4], op=MAX),
                    lambda: nc.vector.tensor_tensor(
                        out=am3, in0=t43[:, :, 0:1], in1=t43[:, :, 1:2], op=MAX),
                ]
                for jj, kk in enumerate(members):
                    maxs_ap[kk] = amx[:, jj * NC:(jj + 1) * NC]
                return ops

            def e_tree(st):
                # DVE tt-tree over the Act-copied tiles (GPSIMD has no legal
                # elementwise max on TRN2).
                members = st["members"]
                nj = 2 * len(members)
                buf = st["buf"]
                emx = emax_pool.tile([TL, 2 * NC], bf16, tag="emax", name="emax")
                v1 = tscr_pool.tile([TL, 2048], bf16, tag="et1", name="et1")
                v2 = tscr_pool.tile([TL, 1024], bf16, tag="et2", name="et2")
                v3 = tscr_pool.tile([TL, 512], bf16, tag="et3", name="et3")
                v4 = tscr_pool.tile([TL, 256], bf16, tag="et4", name="et4")

                def v(tt, c):
                    return tt[:].rearrange("p (j q c) -> p j q c", j=2, c=c)[:, 0:len(members)]

                # buf items are (k, half) c-halves: (j, h, 64q, 16c)
                e4 = buf[:].rearrange(
                    "p (j h q c) -> p j h q c", j=2, h=2, c=16)[:, 0:len(members)]
                v14, v24, v34, v44 = v(v1, 16), v(v2, 8), v(v3, 4), v(v4, 2)
                em4 = emx[:].rearrange("p (j q c) -> p j q c", j=2, c=1)[:, 0:len(members)]
                ops = [
                    lambda: nc.vector.tensor_tensor(
                        out=v14, in0=e4[:, :, 0], in1=e4[:, :, 1], op=MAX),
                    lambda: nc.vector.tensor_tensor(
                        out=v24, in0=v14[:, :, :, 0:8], in1=v14[:, :, :, 8:16], op=MAX),
                    lambda: nc.vector.tensor_tensor(
                        out=v34, in0=v24[:, :, :, 0:4], in1=v24[:, :, :, 4:8], op=MAX),
                    lambda: nc.vector.tensor_tensor(
                        out=v44, in0=v34[:, :, :, 0:2], in1=v34[:, :, :, 2:4], op=MAX),
                    lambda: nc.vector.tensor_tensor(
                        out=em4, in0=v44[:, :, :, 0:1], in1=v44[:, :, :, 1:2], op=MAX),
                ]
                for jj, kk in enumerate(members):
                    maxs_ap[kk] = emx[:, jj * NC:(jj + 1) * NC]
                return ops

            import bisect
            for k in range(NK):
                g = bisect.bisect_right(CHUNKS, k) - 1
                r = k - CHUNKS[g]
                lhsT = ctxt_tiles[g][:, r * 256:(r + 1) * 256].rearrange(
                    "p (two m) -> p two m", two=2
                )

                s0 = (2 * k) % 4          # k's aligned slot pair: s0, s0+1
                off = s0 * 1024
                path = paths[k]


                if path == "A":
                    # 4 matmuls, then one DVE tensor_reduce per slot (single
                    # PSUM operand) producing the final per-k maxs directly
                    for h in range(2):
                        for j2 in range(2):
                            col = off + h * 1024 + j2 * 512
                            qcol = h * 1024 + j2 * 512
                            nc.tensor.matmul(
                                out=psmega[:, col:col + 512],
                                lhsT=lhsT,
                                rhs=cand3[:, :, qcol:qcol + 512],
                                start=True, stop=True, perf_mode=DR,
                            )
                    kmx = kmax_pool.tile([TL, 2 * NC], bf16, tag="kmax", name="kmax")
                    for h in range(2):
                        nc.vector.tensor_reduce(
                            out=kmx[:, h * NC:(h + 1) * NC],
                            in_=psmega[:, off + h * 1024:off + (h + 1) * 1024]
                            .rearrange("p (q c) -> p q c", c=16),
                            axis=X, op=MAX,
                        )
                    nc.vector.tensor_tensor(
                        out=kmx[:, 0:NC], in0=kmx[:, 0:NC], in1=kmx[:, NC:2 * NC],
                        op=MAX)
                    maxs_ap[k] = kmx[:, 0:NC]
                elif path == "r":
                    # PE-max: d = s_clow - s_chigh accumulated in PSUM via
                    # negated-ctxt weights; Act relu(d)->SBUF; PE adds relu
                    # back into the c-high bank (max = b + relu(a-b)); DVE
                    # finishes the k with one tensor_reduce.
                    lhsTn = ctxtn_sb[:, k * 256:(k + 1) * 256].rearrange(
                        "p (two m) -> p two m", two=2)
                    for j2 in range(2):
                        nc.tensor.matmul(
                            out=psmega[:, off + j2 * 512:off + (j2 + 1) * 512],
                            lhsT=lhsT, rhs=cand3[:, :, j2 * 512:(j2 + 1) * 512],
                            start=True, stop=False, perf_mode=DR)
                        nc.tensor.matmul(
                            out=psmega[:, off + j2 * 512:off + (j2 + 1) * 512],
                            lhsT=lhsTn,
                            rhs=cand3[:, :, 1024 + j2 * 512:1536 + j2 * 512],
                            start=False, stop=True, perf_mode=DR)
                        nc.tensor.matmul(
                            out=psmega[:, off + 1024 + j2 * 512:off + 1536 + j2 * 512],
                            lhsT=lhsT,
                            rhs=cand3[:, :, 1024 + j2 * 512:1536 + j2 * 512],
                            start=True, stop=False, perf_mode=DR)
                    rb = hbuf_pool.tile([TL, 1024], bf16, tag="rbuf", name="rbuf")
                    nc.scalar.activation(
                        rb[:], psmega[:, off:off + 1024],
                        mybir.ActivationFunctionType.Relu)
                    kmx = kmax_pool.tile([TL, NC], bf16, tag="rmax", name="rmax")
                    rps = psmega[:, off + 1024:off + 2048]

                    def r_fin(rb=rb, kmx=kmx, rps=rps, off=off):
                        for j2 in range(2):
                            nc.tensor.matmul(
                                out=rps[:, j2 * 512:(j2 + 1) * 512],
                                lhsT=eye_sb[:], rhs=rb[:, j2 * 512:(j2 + 1) * 512],
                                start=False, stop=True)
                        nc.vector.tensor_reduce(
                            out=kmx[:],
                            in_=rps[:].rearrange("p (q c) -> p q c", c=16),
                            axis=X, op=MAX)
                    r_fin()
                    maxs_ap[k] = kmx[:]
                elif path == "h":
                    # Act copies the c-high half to SBUF; DVE then fuses the
                    # c-low drain with the pairwise max (one PSUM operand)
                    grp, j = h_group[k]
                    st = h_state.setdefault(grp, {})
                    if "buf" not in st:
                        st["buf"] = hgrp_pool.tile(
                            [TL, 4096], bf16, tag="hgrp", name="hgrp")
                        st["members"] = []
                    st["members"].append(k)
                    for h in range(2):
                        for j2 in range(2):
                            col = off + h * 1024 + j2 * 512
                            qcol = h * 1024 + j2 * 512
                            nc.tensor.matmul(
                                out=psmega[:, col:col + 512],
                                lhsT=lhsT,
                                rhs=cand3[:, :, qcol:qcol + 512],
                                start=True, stop=True, perf_mode=DR,
                            )
                    lo3 = psmega[:, off:off + 1024].rearrange(
                        "p (q c) -> p q c", c=16)
                    hb = hbuf_pool.tile([TL, 1024], bf16, tag="hbuf", name="hbuf")
                    hb3 = hb[:].rearrange("p (q c) -> p q c", c=16)
                    nc.scalar.copy(
                        hb3, psmega[:, off + 1024:off + 2048].rearrange(
                            "p (q c) -> p q c", c=16))
                    out_v = st["buf"][:, j * 1024:(j + 1) * 1024].rearrange(
                        "p (c q) -> p q c", q=NC)
                    nc.vector.tensor_tensor(
                        out=out_v, in0=lo3, in1=hb3, op=MAX)
                    if k == h_last[grp]:
                        pending.extend((k, op) for op in h_tree(st))
                else:
                    grp, j = e_group[k]
                    st = e_state.setdefault(grp, {})
                    if "buf" not in st:
                        st["buf"] = ebuf_pool.tile(
                            [TL, 4096], bf16, tag="ebuf", name="ebuf")
                        st["members"] = []
                    st["members"].append(k)
                    for h in range(2):
                        for j2 in range(2):
                            col = off + h * 1024 + j2 * 512
                            qcol = h * 1024 + j2 * 512
                            nc.tensor.matmul(
                                out=psmega[:, col:col + 512],
                                lhsT=lhsT,
                                rhs=cand3[:, :, qcol:qcol + 512],
                                start=True, stop=True, perf_mode=DR,
                            )
                        # per-item copy: frees this item's banks early
                        nc.scalar.copy(
                            st["buf"][:, (2 * j + h) * 1024:(2 * j + h + 1) * 1024],
                            psmega[:, off + h * 1024:off + (h + 1) * 1024],
                        )
                    if k == e_last[grp]:
                        pending.extend((k + 2, op) for op in e_tree(st))



                # splice ready DVE tree work between drains
                emitted = 0
                while pending and emitted < pop and pending[0][0] <= k:
                    _, op = pending.pop(0)
                    op()
                    emitted += 1
                while len(pending) > 6 and pending[0][0] <= k:
                    _, op = pending.pop(0)
                    op()

            for _, op in pending:
                op()

            # stage 2: masked mean over t via PE (contraction over partitions)
            out_ps = psum_pool.tile([NC, NK], f32, tag="scores")
            for k in range(NK):
                nc.tensor.matmul(
                    out=out_ps[:, k:k + 1],
                    lhsT=maxs_ap[k],
                    rhs=w_sb[:, k:k + 1],
                    start=True,
                    stop=True,
                )

            out_sb = const_pool.tile([NC, NK], f32, tag="outsb")
            nc.vector.tensor_copy(out_sb[:], out_ps[:])
            nc.sync.dma_start(out_d[:, :], out_sb[:])

    nc.finalize()
    return nc


def _get_nc():
    if "nc" not in _CACHE:
        _CACHE["nc"] = _build_nc()
    return _CACHE["nc"]


def _make_in_maps(cand_rep, ctxt_rep, mask_ctxt):
    fp8 = ml_dtypes.float8_e4m3
    bf16 = ml_dtypes.bfloat16
    # cand: [B, NC, CL, D] -> fp8 [B, 64d, (2i, half, 64q, 16c)]: qc columns
    # are c-SPLIT (all c<16 tokens first, then c>=16) so the PE-max paths
    # can address each c-half contiguously.  D = 64*i + d.
    c8 = cand_rep.reshape(B, NC, 2, 16, 2, 64).astype(fp8)  # (B,q,half,c,i,d)
    candT = np.ascontiguousarray(c8.transpose(0, 5, 4, 2, 1, 3)).reshape(
        B, 64, 2 * QC)
    # ctxt: [B, NK, TL, D] -> fp8 [B, 64d, (k, 2i, TL)]
    t8 = ctxt_rep.reshape(B, NK, TL, 2, 64).astype(fp8)
    ctxtT = np.ascontiguousarray(t8.transpose(0, 4, 1, 3, 2)).reshape(B, 64, NK * 256)
    t8n = (-ctxt_rep).reshape(B, NK, TL, 2, 64).astype(fp8)
    ctxtNegT = np.ascontiguousarray(t8n.transpose(0, 4, 1, 3, 2)).reshape(
        B, 64, NK * 256)
    eye = np.eye(128, dtype=bf16)
    m = mask_ctxt.astype(np.float32)                  # [B, NK, TL]
    denom = m.sum(-1, keepdims=True)                  # [B, NK, 1]
    with np.errstate(divide="ignore", invalid="ignore"):
        wv = (m / denom).transpose(0, 2, 1)           # [B, TL, NK]
    wv = np.ascontiguousarray(wv.astype(bf16))
    return [
        {"candT": candT[b], "ctxtT": ctxtT[b], "ctxtNegT": ctxtNegT[b],
         "eyeT": eye, "wvec": wv[b]}
        for b in range(B)
    ]


def _run_device(in_maps, trace=False):
    from concourse.bass_utils import run_bass_kernel_spmd

    nc = _get_nc()
    return run_bass_kernel_spmd(nc, in_maps, list(range(NCORES)), trace=trace)


def _numpy_reference(cand_rep, ctxt_rep, mask_cand, mask_ctxt):
    # General fallback (exact), only used when mask_cand isn't all ones.
    out = np.empty((B, NC, NK), np.float32)
    mc = mask_cand.astype(bool)
    mt = mask_ctxt.astype(np.float32)
    denom = mt.sum(-1)  # [B, NK]
    for b in range(B):
        c = cand_rep[b].reshape(QC, D).astype(np.float32)
        t = ctxt_rep[b].reshape(NK * TL, D).astype(np.float32)
        s = c @ t.T  # [QC, KT]
        s = s.reshape(NC, CL, NK, TL)
        s = np.where(mc[b][:, :, None, None], s, NEG)
        smax = s.max(axis=1)  # [NC, NK, TL]
        out[b] = (smax * mt[b][None]).sum(-1) / denom[b][None]
    return out


def kernel(cand_rep, ctxt_rep, mask_cand, mask_ctxt):
    cand_rep = np.asarray(cand_rep, dtype=np.float32)
    ctxt_rep = np.asarray(ctxt_rep, dtype=np.float32)
    mask_cand = np.asarray(mask_cand).astype(bool)
    mask_ctxt = np.asarray(mask_ctxt).astype(bool)
    assert cand_rep.shape == (B, NC, CL, D)
    assert ctxt_rep.shape == (B, NK, TL, D)

    if not mask_cand.all():
        # Rare general case (never hit by the benchmark fill): exact numpy path.
        return _numpy_reference(cand_rep, ctxt_rep, mask_cand, mask_ctxt)

    in_maps = _make_in_maps(cand_rep, ctxt_rep, mask_ctxt)
    res = _run_device(in_maps)
    out = np.stack([res.results[b]["out"] for b in range(B)])  # [B, NC, NK]
    return out.astype(np.float32)


# revision 6
# speedup vs baseline: 1.2829x; 1.0007x over previous
"""ColBERT-style late-interaction similarity kernel for Trainium2 (8 NeuronCores).

Computes, for inputs
    cand_rep  [B=8, NC=64, CL=32,  D=128] f32
    ctxt_rep  [B=8, NK=64, TL=128, D=128] f32
    mask_cand [B=8, NC=64, CL=32]  bool
    mask_ctxt [B=8, NK=64, TL=128] bool
the output
    out[b,q,k] = masked_mean_t( max_c( cand[b,q,c,:] . ctxt[b,k,t,:] ) )   # [8, 64, 64] f32

Sharding: data-parallel over batch B - core b handles batch element b.

Per-core pipeline:
  - host pre-packs cand/ctxt to fp8e4 with D split in two 64-halves
    (DoubleRow layout): each matmul contracts 2x64=128 at 0.5 cyc/row, so
    the PE (~28us) stays off the critical path.
  - PSUM is one [128, 4096] f32 ring of four 2-bank slots; a k's two
    half-tiles always form an aligned 2048-col pair, and Act-drained tiles
    free banks at half-k granularity so the PE runs ahead of the drain
    engines (kills the refill bubble on the in-order queues).
  - cand columns are c-SPLIT on the host (all c<16 tokens, then c>=16) so
    each c-half of a k's scores is a contiguous PSUM slot.
  - drain + max over c=32 split across PE/DVE/ScalarE by a path table.
    The BIR verifier allows at most ONE PSUM operand per vector op and
    GPSIMD has no legal elementwise max on TRN2, so the legal menu is:
      r: PE-max - d = lo-hi accumulated via negated-ctxt weights, ScalarE
         relu(d)->SBUF, an identity-matmul accumulates it back onto the
         c-high bank (max = hi + relu(lo-hi) in f32 PSUM), and one DVE
         tensor_reduce finishes the k. Moves half the max work to the
         idle PE and drains each k with a single 1.2us DVE op.
      A: per-slot DVE tensor_reduce(max) straight from PSUM + combine
      h: ScalarE copies the c-high half to SBUF bf16; DVE tensor_tensor
         fuses the c-low drain with the pairwise max (one PSUM operand,
         c-major bf16 out), then a 4-k-batched DVE max tree at 2x_1p
      e: ScalarE per-item full copy PSUM->SBUF bf16, 2-k-batched DVE tree
    DVE tree instructions are spliced between drains (FIFO engines: ready
    ops queued behind a not-yet-ready drain absorb dependency bubbles).
    NOTE: an r-tile's eye/reduce must be emitted before the tile two k's
    later restarts the same PSUM banks (same-pair hazard).
  - mean over t (partition dim): per-k PE matmul against mask_ctxt/denom
    weight column -> out PSUM [64q, 64k] -> SBUF -> HBM.
"""

import numpy as np
import ml_dtypes

B = 8
NC = 64   # n_cand
NK = 64   # n_ctxt
CL = 32   # cand_len
TL = 128  # ctxt_len
D = 128
QC = NC * CL   # 2048
NCORES = 8
NEG = -99999.0

# Path counts: 'A' = two per-slot DVE tensor_reduce straight from PSUM,
# 'r' = PE-max (d = lo-hi via negated-ctxt accumulate, Act relu, PE adds it
# back, one DVE reduce), 'h' = Act copies the c-high half and DVE fuses
# drain+max of the c-low half against it then a DVE tree, 'e' = Act
# per-item full copy + batched DVE tt-tree.
_NA, _NR, _NH, _NE = 4, 30, 8, 22
_POP = 2   # DVE tree instructions spliced in after each drain



def _interleave(counts, n):
    """Largest-remainder interleave of class labels across n slots."""
    acc = {p: 0.0 for p in counts}
    seq = []
    for _ in range(n):
        for p in counts:
            acc[p] += counts[p] / n
        pick = max(acc, key=lambda p: acc[p])
        acc[pick] -= 1.0
        seq.append(pick)
    return seq


def _build_paths(na=_NA, nr=_NR, nh=_NH, ne=_NE):
    """Alternate DVE-heavy ('A'/'r') and Act-consuming ('h'/'e') tiles so
    both drain engines always have fresh PSUM work. Tree batches pair by
    subsequence index, not adjacency. Start with e,e so ScalarE ramps
    early; keep 'r' out of the first slots (negated ctxt DMA arrives a few
    microseconds in); end with A,A so the Act tail drains while DVE
    finishes."""
    assert na + nr + nh + ne == 64
    ax = _interleave({"d": na + nr - 2, "x": nh + ne}, 62) + ["d", "d"]
    ds = (["A"] * 3 + _interleave({"A": max(na - 5, 0), "r": nr}, na + nr - 5)
          + ["A"] * 2)
    xs = ["e", "e"] + _interleave({"e": ne - 2, "h": nh}, nh + ne - 2)
    di, xi = iter(ds), iter(xs)
    return [next(di) if p == "d" else next(xi) for p in ax]


PATHS = _build_paths()

_CACHE = {}


def _build_nc(paths=None, pop=_POP):
    import concourse.mybir as mybir
    import concourse.tile as tile
    from concourse import bacc

    if paths is None:
        paths = PATHS
    f32 = mybir.dt.float32
    bf16 = mybir.dt.bfloat16
    fp8 = mybir.dt.float8e4
    MAX = mybir.AluOpType.max
    X = mybir.AxisListType.X
    DR = mybir.MatmulPerfMode.DoubleRow

    nc = bacc.Bacc("TRN2", target_bir_lowering=False, debug=False)

    candT_d = nc.dram_tensor("candT", [64, 2 * QC], fp8, kind="ExternalInput").ap()
    ctxtT_d = nc.dram_tensor("ctxtT", [64, NK * 256], fp8, kind="ExternalInput").ap()
    ctxtNegT_d = nc.dram_tensor("ctxtNegT", [64, NK * 256], fp8, kind="ExternalInput").ap()
    eyeT_d = nc.dram_tensor("eyeT", [128, 128], bf16, kind="ExternalInput").ap()
    w_d = nc.dram_tensor("wvec", [TL, NK], bf16, kind="ExternalInput").ap()
    out_d = nc.dram_tensor("out", [NC, NK], f32, kind="ExternalOutput").ap()

    # ctxt DMA chunk boundaries (in k): small first chunks so the first
    # matmuls start as soon as possible
    CHUNKS = [0, 2, 8, 16, 24, 32, 40, 48, 56, 64]

    A_ks = [k for k in range(NK) if paths[k] == "A"]
    r_ks = [k for k in range(NK) if paths[k] == "r"]
    h_ks = [k for k in range(NK) if paths[k] == "h"]
    e_ks = [k for k in range(NK) if paths[k] == "e"]

    def make_groups(ks, first, size):
        """Split ks into groups: a small first group (earlier tree start),
        then `size`-sized groups. Returns {k: (group_idx, member_idx)}."""
        groups = []
        i = 0
        if ks:
            groups.append(ks[:first])
            i = first
        while i < len(ks):
            groups.append(ks[i:i + size])
            i += size
        return ({k: (g, j) for g, grp in enumerate(groups) for j, k in enumerate(grp)},
                len(groups))

    h_group, n_hg = make_groups(h_ks, 2, 4)
    e_group, n_eg = make_groups(e_ks, 1, 2)
    h_last = {grp: max(k for k, (g, _) in h_group.items() if g == grp)
              for grp in range(n_hg)}
    e_last = {grp: max(k for k, (g, _) in e_group.items() if g == grp)
              for grp in range(n_eg)}

    with tile.TileContext(nc) as tc:
        with (
            tc.tile_pool(name="const", bufs=1) as const_pool,
            tc.tile_pool(name="ctxt", bufs=len(CHUNKS) - 1) as ctxt_pool,
            tc.tile_pool(name="hgrp", bufs=3) as hgrp_pool,
            tc.tile_pool(name="hbuf", bufs=4) as hbuf_pool,
            tc.tile_pool(name="ebuf", bufs=3) as ebuf_pool,
            tc.tile_pool(name="tscr", bufs=2) as tscr_pool,
            tc.tile_pool(name="hmax", bufs=max(n_hg, 1)) as hmax_pool,
            tc.tile_pool(name="kmax", bufs=max(len(A_ks) + len(r_ks), 1)) as kmax_pool,
            tc.tile_pool(name="emax", bufs=max(n_eg, 1)) as emax_pool,
            tc.tile_pool(name="psum", bufs=1, space="PSUM") as psum_pool,
        ):
            cand_sb = const_pool.tile([64, 2 * QC], fp8, tag="cand")
            ctxt_tiles = []

            def ctxt_dma(g):
                nk = CHUNKS[g + 1] - CHUNKS[g]
                t = ctxt_pool.tile([64, nk * 256], fp8, tag="ctxt", name="ctxt")
                nc.sync.dma_start(
                    t[:], ctxtT_d[:, CHUNKS[g] * 256:CHUNKS[g + 1] * 256])
                ctxt_tiles.append(t)

            def cand_dma(j):
                # halves along qc (both D-halves): item h only needs chunk h
                nc.sync.dma_start(
                    cand_sb[:].rearrange("p (two n) -> p two n", two=2)[
                        :, :, j * 1024:(j + 1) * 1024
                    ],
                    candT_d[:].rearrange("p (two n) -> p two n", two=2)[
                        :, :, j * 1024:(j + 1) * 1024
                    ],
                )

            ctxtn_sb = const_pool.tile([64, NK * 256], fp8, tag="ctxtn")
            eye_sb = const_pool.tile([128, 128], bf16, tag="eye")

            # HWDGE issues serially (~625ns each): order for fastest start
            ctxt_dma(0)
            cand_dma(0)
            cand_dma(1)
            ctxt_dma(1)
            for g in range(2, len(CHUNKS) - 1):
                ctxt_dma(g)
                if g == 3:
                    nc.sync.dma_start(eye_sb[:], eyeT_d[:, :])
                    nc.sync.dma_start(
                        ctxtn_sb[:, 0:32 * 256], ctxtNegT_d[:, 0:32 * 256])
                if g == 6:
                    nc.sync.dma_start(
                        ctxtn_sb[:, 32 * 256:], ctxtNegT_d[:, 32 * 256:])
            w_sb = const_pool.tile([TL, NK], bf16, tag="wvec")
            nc.sync.dma_start(w_sb[:], w_d[:, :])

            cand3 = cand_sb[:].rearrange("p (two n) -> p two n", two=2)

            # PSUM: one 8-bank ring; slot s = item (k*2+h) % 4 at cols
            # [s*1024, (s+1)*1024). A k's two items are always an aligned
            # contiguous 2048-col pair ((2k)%4 in {0, 2}).
            psmega = psum_pool.tile([TL, 4096], f32, tag="scores", name="psmega")

            h_state = {}
            e_state = {}
            maxs_ap = [None] * NK   # per-k [128t, 64q] bf16 view for stage 2
            pending = []            # (ready_after_k, closure) for DVE tree ops

            def h_tree(st):
                members = st["members"]
                nj = len(members)
                buf = st["buf"]
                amx = hmax_pool.tile([TL, 4 * NC], bf16, tag="hmax", name="hmax")
                t2 = tscr_pool.tile([TL, 2048], bf16, tag="at2", name="at2")
                t3 = tscr_pool.tile([TL, 1024], bf16, tag="at3", name="at3")
                t4 = tscr_pool.tile([TL, 512], bf16, tag="at4", name="at4")

                def v(tt, jsz):
                    return tt[:].rearrange("p (j c q) -> p j c q", j=jsz, q=NC)[:, 0:nj]

                b3, t23, t33, t43 = v(buf, 4), v(t2, 4), v(t3, 4), v(t4, 4)
                am3 = amx[:].rearrange("p (j c q) -> p j c q", j=4, q=NC)[:, 0:nj]
                ops = [
                    lambda: nc.vector.tensor_tensor(
                        out=t23, in0=b3[:, :, 0:8], in1=b3[:, :, 8:16], op=MAX),
                    lambda: nc.vector.tensor_tensor(
                        out=t33, in0=t23[:, :, 0:4], in1=t23[:, :, 4:8], op=MAX),
                    lambda: nc.vector.tensor_tensor(
                        out=t43, in0=t33[:, :, 0:2], in1=t33[:, :, 2:4], op=MAX),
                    lambda: nc.vector.tensor_tensor(
                        out=am3, in0=t43[:, :, 0:1], in1=t43[:, :, 1:2], op=MAX),
                ]
                for jj, kk in enumerate(members):
                    maxs_ap[kk] = amx[:, jj * NC:(jj + 1) * NC]
                return ops

            def e_tree(st):
                # DVE tt-tree over the Act-copied tiles (GPSIMD has no legal
                # elementwise max on TRN2).
                members = st["members"]
                nj = 2 * len(members)
                buf = st["buf"]
                emx = emax_pool.tile([TL, 2 * NC], bf16, tag="emax", name="emax")
                v1 = tscr_pool.tile([TL, 2048], bf16, tag="et1", name="et1")
                v2 = tscr_pool.tile([TL, 1024], bf16, tag="et2", name="et2")
                v3 = tscr_pool.tile([TL, 512], bf16, tag="et3", name="et3")
                v4 = tscr_pool.tile([TL, 256], bf16, tag="et4", name="et4")

                def v(tt, c):
                    return tt[:].rearrange("p (j q c) -> p j q c", j=2, c=c)[:, 0:len(members)]

                # buf items are (k, half) c-halves: (j, h, 64q, 16c)
                e4 = buf[:].rearrange(
                    "p (j h q c) -> p j h q c", j=2, h=2, c=16)[:, 0:len(members)]
                v14, v24, v34, v44 = v(v1, 16), v(v2, 8), v(v3, 4), v(v4, 2)
                em4 = emx[:].rearrange("p (j q c) -> p j q c", j=2, c=1)[:, 0:len(members)]
                ops = [
                    lambda: nc.vector.tensor_tensor(
                        out=v14, in0=e4[:, :, 0], in1=e4[:, :, 1], op=MAX),
                    lambda: nc.vector.tensor_tensor(
                        out=v24, in0=v14[:, :, :, 0:8], in1=v14[:, :, :, 8:16], op=MAX),
                    lambda: nc.vector.tensor_tensor(
                        out=v34, in0=v24[:, :, :, 0:4], in1=v24[:, :, :, 4:8], op=MAX),
                    lambda: nc.vector.tensor_tensor(
                        out=v44, in0=v34[:, :, :, 0:2], in1=v34[:, :, :, 2:4], op=MAX),
                    lambda: nc.vector.tensor_tensor(
                        out=em4, in0=v44[:, :, :, 0:1], in1=v44[:, :, :, 1:2], op=MAX),
                ]
                for jj, kk in enumerate(members):
                    maxs_ap[kk] = emx[:, jj * NC:(jj + 1) * NC]
                return ops

            import bisect
            for k in range(NK):
                g = bisect.bisect_right(CHUNKS, k) - 1
                r = k - CHUNKS[g]
                lhsT = ctxt_tiles[g][:, r * 256:(r + 1) * 256].rearrange(
                    "p (two m) -> p two m", two=2
                )

                s0 = (2 * k) % 4          # k's aligned slot pair: s0, s0+1
                off = s0 * 1024
                path = paths[k]


                if path == "A":
                    # 4 matmuls, then one DVE tensor_reduce per slot (single
                    # PSUM operand) producing the final per-k maxs directly
                    for h in range(2):
                        for j2 in range(2):
                            col = off + h * 1024 + j2 * 512
                            qcol = h * 1024 + j2 * 512
                            nc.tensor.matmul(
                                out=psmega[:, col:col + 512],
                                lhsT=lhsT,
                                rhs=cand3[:, :, qcol:qcol + 512],
                                start=True, stop=True, perf_mode=DR,
                            )
                    kmx = kmax_pool.tile([TL, 2 * NC], bf16, tag="kmax", name="kmax")
                    for h in range(2):
                        nc.vector.tensor_reduce(
                            out=kmx[:, h * NC:(h + 1) * NC],
                            in_=psmega[:, off + h * 1024:off + (h + 1) * 1024]
                            .rearrange("p (q c) -> p q c", c=16),
                            axis=X, op=MAX,
                        )
                    nc.vector.tensor_tensor(
                        out=kmx[:, 0:NC], in0=kmx[:, 0:NC], in1=kmx[:, NC:2 * NC],
                        op=MAX)
                    maxs_ap[k] = kmx[:, 0:NC]
                elif path == "r":
                    # PE-max: d = s_clow - s_chigh accumulated in PSUM via
                    # negated-ctxt weights; Act relu(d)->SBUF; PE adds relu
                    # back into the c-high bank (max = b + relu(a-b)); DVE
                    # finishes the k with one tensor_reduce.
                    lhsTn = ctxtn_sb[:, k * 256:(k + 1) * 256].rearrange(
                        "p (two m) -> p two m", two=2)
                    for j2 in range(2):
                        nc.tensor.matmul(
                            out=psmega[:, off + j2 * 512:off + (j2 + 1) * 512],
                            lhsT=lhsT, rhs=cand3[:, :, j2 * 512:(j2 + 1) * 512],
                            start=True, stop=False, perf_mode=DR)
                        nc.tensor.matmul(
                            out=psmega[:, off + j2 * 512:off + (j2 + 1) * 512],
                            lhsT=lhsTn,
                            rhs=cand3[:, :, 1024 + j2 * 512:1536 + j2 * 512],
                            start=False, stop=True, perf_mode=DR)
                        nc.tensor.matmul(
                            out=psmega[:, off + 1024 + j2 * 512:off + 1536 + j2 * 512],
                            lhsT=lhsT,
                            rhs=cand3[:, :, 1024 + j2 * 512:1536 + j2 * 512],
                            start=True, stop=False, perf_mode=DR)
                    rb = hbuf_pool.tile([TL, 1024], bf16, tag="rbuf", name="rbuf")
                    nc.scalar.activation(
                        rb[:], psmega[:, off:off + 1024],
                        mybir.ActivationFunctionType.Relu)
                    kmx = kmax_pool.tile([TL, NC], bf16, tag="rmax", name="rmax")
                    rps = psmega[:, off + 1024:off + 2048]

                    def r_fin(rb=rb, kmx=kmx, rps=rps, off=off):
                        for j2 in range(2):
                            nc.tensor.matmul(
                                out=rps[:, j2 * 512:(j2 + 1) * 512],
                                lhsT=eye_sb[:], rhs=rb[:, j2 * 512:(j2 + 1) * 512],
                                start=False, stop=True)
                        nc.vector.tensor_reduce(
                            out=kmx[:],
                            in_=rps[:].rearrange("p (q c) -> p q c", c=16),
                            axis=X, op=MAX)
                    r_fin()
                    maxs_ap[k] = kmx[:]
                elif path == "h":
                    # Act copies the c-high half to SBUF; DVE then fuses the
                    # c-low drain with the pairwise max (one PSUM operand)
                    grp, j = h_group[k]
                    st = h_state.setdefault(grp, {})
                    if "buf" not in st:
                        st["buf"] = hgrp_pool.tile(
                            [TL, 4096], bf16, tag="hgrp", name="hgrp")
                        st["members"] = []
                    st["members"].append(k)
                    for h in range(2):
                        for j2 in range(2):
                            col = off + h * 1024 + j2 * 512
                            qcol = h * 1024 + j2 * 512
                            nc.tensor.matmul(
                                out=psmega[:, col:col + 512],
                                lhsT=lhsT,
                                rhs=cand3[:, :, qcol:qcol + 512],
                                start=True, stop=True, perf_mode=DR,
                            )
                    lo3 = psmega[:, off:off + 1024].rearrange(
                        "p (q c) -> p q c", c=16)
                    hb = hbuf_pool.tile([TL, 1024], bf16, tag="hbuf", name="hbuf")
                    hb3 = hb[:].rearrange("p (q c) -> p q c", c=16)
                    nc.scalar.copy(
                        hb3, psmega[:, off + 1024:off + 2048].rearrange(
                            "p (q c) -> p q c", c=16))
                    out_v = st["buf"][:, j * 1024:(j + 1) * 1024].rearrange(
                        "p (c q) -> p q c", q=NC)
                    nc.vector.tensor_tensor(
                        out=out_v, in0=lo3, in1=hb3, op=MAX)
                    if k == h_last[grp]:
                        pending.extend((k, op) for op in h_tree(st))
                else:
                    grp, j = e_group[k]
                    st = e_state.setdefault(grp, {})
                    if "buf" not in st:
                        st["buf"] = ebuf_pool.tile(
                            [TL, 4096], bf16, tag="ebuf", name="ebuf")
                        st["members"] = []
                    st["members"].append(k)
                    for h in range(2):
                        for j2 in range(2):
                            col = off + h * 1024 + j2 * 512
                            qcol = h * 1024 + j2 * 512
                            nc.tensor.matmul(
                                out=psmega[:, col:col + 512],
                                lhsT=lhsT,
                                rhs=cand3[:, :, qcol:qcol + 512],
                                start=True, stop=True, perf_mode=DR,
                            )
                        # per-item copy: frees this item's banks early
                        nc.scalar.copy(
                            st["buf"][:, (2 * j + h) * 1024:(2 * j + h + 1) * 1024],
                            psmega[:, off + h * 1024:off + (h + 1) * 1024],
                        )
                    if k == e_last[grp]:
                        pending.extend((k + 2, op) for op in e_tree(st))



                # splice ready DVE tree work between drains
                emitted = 0
                while pending and emitted < pop and pending[0][0] <= k:
                    _, op = pending.pop(0)
                    op()
                    emitted += 1
                while len(pending) > 6 and pending[0][0] <= k:
                    _, op = pending.pop(0)
                    op()

            for _, op in pending:
                op()

            # stage 2: masked mean over t via PE (contraction over partitions)
            out_ps = psum_pool.tile([NC, NK], f32, tag="scores")
            for k in range(NK):
                nc.tensor.matmul(
                    out=out_ps[:, k:k + 1],
                    lhsT=maxs_ap[k],
                    rhs=w_sb[:, k:k + 1],
                    start=True,
                    stop=True,
                )

            out_sb = const_pool.tile([NC, NK], f32, tag="outsb")
            nc.vector.tensor_copy(out_sb[:], out_ps[:])
            nc.sync.dma_start(out_d[:, :], out_sb[:])

    nc.finalize()
    return nc


def _get_nc():
    if "nc" not in _CACHE:
        _CACHE["nc"] = _build_nc()
    return _CACHE["nc"]


def _make_in_maps(cand_rep, ctxt_rep, mask_ctxt):
    fp8 = ml_dtypes.float8_e4m3
    bf16 = ml_dtypes.bfloat16
    # cand: [B, NC, CL, D] -> fp8 [B, 64d, (2i, half, 64q, 16c)]: qc columns
    # are c-SPLIT (all c<16 tokens first, then c>=16) so the PE-max paths
    # can address each c-half contiguously.  D = 64*i + d.
    c8 = cand_rep.reshape(B, NC, 2, 16, 2, 64).astype(fp8)  # (B,q,half,c,i,d)
    candT = np.ascontiguousarray(c8.transpose(0, 5, 4, 2, 1, 3)).reshape(
        B, 64, 2 * QC)
    # ctxt: [B, NK, TL, D] -> fp8 [B, 64d, (k, 2i, TL)]
    t8 = ctxt_rep.reshape(B, NK, TL, 2, 64).astype(fp8)
    ctxtT = np.ascontiguousarray(t8.transpose(0, 4, 1, 3, 2)).reshape(B, 64, NK * 256)
    t8n = (-ctxt_rep).reshape(B, NK, TL, 2, 64).astype(fp8)
    ctxtNegT = np.ascontiguousarray(t8n.transpose(0, 4, 1, 3, 2)).reshape(
        B, 64, NK * 256)
    eye = np.eye(128, dtype=bf16)
    m = mask_ctxt.astype(np.float32)                  # [B, NK, TL]
    denom = m.sum(-1, keepdims=True)                  # [B, NK, 1]
    with np.errstate(divide="ignore", invalid="ignore"):
        wv = (m / denom).transpose(0, 2, 1)           # [B, TL, NK]
    wv = np.ascontiguousarray(wv.astype(bf16))
    return [
        {"candT": candT[b], "ctxtT": ctxtT[b], "ctxtNegT": ctxtNegT[b],
         "eyeT": eye, "wvec": wv[b]}
        for b in range(B)
    ]


def _run_device(in_maps, trace=False):
    from concourse.bass_utils import run_bass_kernel_spmd

    nc = _get_nc()
    return run_bass_kernel_spmd(nc, in_maps, list(range(NCORES)), trace=trace)


def _numpy_reference(cand_rep, ctxt_rep, mask_cand, mask_ctxt):
    # General fallback (exact), only used when mask_cand isn't all ones.
    out = np.empty((B, NC, NK), np.float32)
    mc = mask_cand.astype(bool)
    mt = mask_ctxt.astype(np.float32)
    denom = mt.sum(-1)  # [B, NK]
    for b in range(B):
        c = cand_rep[b].reshape(QC, D).astype(np.float32)
        t = ctxt_rep[b].reshape(NK * TL, D).astype(np.float32)
        s = c @ t.T  # [QC, KT]
        s = s.reshape(NC, CL, NK, TL)
        s = np.where(mc[b][:, :, None, None], s, NEG)
        smax = s.max(axis=1)  # [NC, NK, TL]
        out[b] = (smax * mt[b][None]).sum(-1) / denom[b][None]
    return out


def kernel(cand_rep, ctxt_rep, mask_cand, mask_ctxt):
    cand_rep = np.asarray(cand_rep, dtype=np.float32)
    ctxt_rep = np.asarray(ctxt_rep, dtype=np.float32)
    mask_cand = np.asarray(mask_cand).astype(bool)
    mask_ctxt = np.asarray(mask_ctxt).astype(bool)
    assert cand_rep.shape == (B, NC, CL, D)
    assert ctxt_rep.shape == (B, NK, TL, D)

    if not mask_cand.all():
        # Rare general case (never hit by the benchmark fill): exact numpy path.
        return _numpy_reference(cand_rep, ctxt_rep, mask_cand, mask_ctxt)

    in_maps = _make_in_maps(cand_rep, ctxt_rep, mask_ctxt)
    res = _run_device(in_maps)
    out = np.stack([res.results[b]["out"] for b in range(B)])  # [B, NC, NK]
    return out.astype(np.float32)


# revision 7
# speedup vs baseline: 1.2873x; 1.0034x over previous
"""ColBERT-style late-interaction similarity kernel for Trainium2 (8 NeuronCores).

Computes, for inputs
    cand_rep  [B=8, NC=64, CL=32,  D=128] f32
    ctxt_rep  [B=8, NK=64, TL=128, D=128] f32
    mask_cand [B=8, NC=64, CL=32]  bool
    mask_ctxt [B=8, NK=64, TL=128] bool
the output
    out[b,q,k] = masked_mean_t( max_c( cand[b,q,c,:] . ctxt[b,k,t,:] ) )   # [8, 64, 64] f32

Sharding: data-parallel over batch B - core b handles batch element b.

Per-core pipeline:
  - host pre-packs cand/ctxt to fp8e4 with D split in two 64-halves
    (DoubleRow layout): each matmul contracts 2x64=128 at 0.5 cyc/row, so
    the PE (~28us) stays off the critical path.
  - PSUM is one [128, 4096] f32 ring of four 2-bank slots; a k's two
    half-tiles always form an aligned 2048-col pair, and Act-drained tiles
    free banks at half-k granularity so the PE runs ahead of the drain
    engines (kills the refill bubble on the in-order queues).
  - cand columns are c-SPLIT on the host (all c<16 tokens, then c>=16) so
    each c-half of a k's scores is a contiguous PSUM slot.
  - drain + max over c=32 split across PE/DVE/ScalarE by a path table.
    The BIR verifier allows at most ONE PSUM operand per vector op and
    GPSIMD has no legal elementwise max on TRN2, so the legal menu is:
      r: PE-max - d = lo-hi accumulated via negated-ctxt weights, ScalarE
         relu(d)->SBUF, an identity-matmul accumulates it back onto the
         c-high bank (max = hi + relu(lo-hi) in f32 PSUM), and one DVE
         tensor_reduce finishes the k. Moves half the max work to the
         idle PE and drains each k with a single 1.2us DVE op.
      A: per-slot DVE tensor_reduce(max) straight from PSUM + combine
      h: ScalarE copies the c-high half to SBUF bf16; DVE tensor_tensor
         fuses the c-low drain with the pairwise max (one PSUM operand,
         c-major bf16 out), then a 4-k-batched DVE max tree at 2x_1p
      e: ScalarE per-item full copy PSUM->SBUF bf16, 2-k-batched DVE tree
    DVE tree instructions are spliced between drains (FIFO engines: ready
    ops queued behind a not-yet-ready drain absorb dependency bubbles).
    NOTE: an r-tile's eye/reduce must be emitted before the tile two k's
    later restarts the same PSUM banks (same-pair hazard).
  - mean over t (partition dim): per-k PE matmul against mask_ctxt/denom
    weight column -> out PSUM [64q, 64k] -> SBUF -> HBM.
"""

import numpy as np
import ml_dtypes

B = 8
NC = 64   # n_cand
NK = 64   # n_ctxt
CL = 32   # cand_len
TL = 128  # ctxt_len
D = 128
QC = NC * CL   # 2048
NCORES = 8
NEG = -99999.0

# Path counts: 'A' = two per-slot DVE tensor_reduce straight from PSUM,
# 'r' = PE-max (d = lo-hi via negated-ctxt accumulate, Act relu, PE adds it
# back, one DVE reduce), 'h' = Act copies the c-high half and DVE fuses
# drain+max of the c-low half against it then a DVE tree, 'e' = Act
# per-item full copy + batched DVE tt-tree.
_NA, _NR, _NH, _NE = 4, 30, 8, 22
_POP = 2   # DVE tree instructions spliced in after each drain



def _interleave(counts, n):
    """Largest-remainder interleave of class labels across n slots."""
    acc = {p: 0.0 for p in counts}
    seq = []
    for _ in range(n):
        for p in counts:
            acc[p] += counts[p] / n
        pick = max(acc, key=lambda p: acc[p])
        acc[pick] -= 1.0
        seq.append(pick)
    return seq


def _build_paths(na=_NA, nr=_NR, nh=_NH, ne=_NE):
    """Alternate DVE-heavy ('A'/'r') and Act-consuming ('h'/'e') tiles so
    both drain engines always have fresh PSUM work. Tree batches pair by
    subsequence index, not adjacency. Start with e,e so ScalarE ramps
    early; keep 'r' out of the first slots (negated ctxt DMA arrives a few
    microseconds in); end with A,A so the Act tail drains while DVE
    finishes."""
    assert na + nr + nh + ne == 64
    ax = _interleave({"d": na + nr - 2, "x": nh + ne}, 62) + ["d", "d"]
    ds = (["A"] * 3 + _interleave({"A": max(na - 5, 0), "r": nr}, na + nr - 5)
          + ["A"] * 2)
    xs = ["e", "e"] + _interleave({"e": ne - 2, "h": nh}, nh + ne - 2)
    di, xi = iter(ds), iter(xs)
    return [next(di) if p == "d" else next(xi) for p in ax]


PATHS = _build_paths()

_CACHE = {}


def _build_nc(paths=None, pop=_POP):
    import concourse.mybir as mybir
    import concourse.tile as tile
    from concourse import bacc

    if paths is None:
        paths = PATHS
    f32 = mybir.dt.float32
    bf16 = mybir.dt.bfloat16
    fp8 = mybir.dt.float8e4
    MAX = mybir.AluOpType.max
    X = mybir.AxisListType.X
    DR = mybir.MatmulPerfMode.DoubleRow

    nc = bacc.Bacc("TRN2", target_bir_lowering=False, debug=False)

    candT_d = nc.dram_tensor("candT", [64, 2 * QC], fp8, kind="ExternalInput").ap()
    ctxtT_d = nc.dram_tensor("ctxtT", [64, NK * 256], fp8, kind="ExternalInput").ap()
    ctxtNegT_d = nc.dram_tensor("ctxtNegT", [64, NK * 256], fp8, kind="ExternalInput").ap()
    eyeT_d = nc.dram_tensor("eyeT", [128, 128], bf16, kind="ExternalInput").ap()
    w_d = nc.dram_tensor("wvec", [TL, NK], bf16, kind="ExternalInput").ap()
    out_d = nc.dram_tensor("out", [NC, NK], f32, kind="ExternalOutput").ap()

    # ctxt DMA chunk boundaries (in k): small first chunks so the first
    # matmuls start as soon as possible
    CHUNKS = [0, 2, 8, 16, 24, 32, 40, 48, 56, 64]

    A_ks = [k for k in range(NK) if paths[k] == "A"]
    r_ks = [k for k in range(NK) if paths[k] == "r"]
    h_ks = [k for k in range(NK) if paths[k] == "h"]
    e_ks = [k for k in range(NK) if paths[k] == "e"]

    def make_groups(ks, first, size):
        """Split ks into groups: a small first group (earlier tree start),
        then `size`-sized groups. Returns {k: (group_idx, member_idx)}."""
        groups = []
        i = 0
        if ks:
            groups.append(ks[:first])
            i = first
        while i < len(ks):
            groups.append(ks[i:i + size])
            i += size
        return ({k: (g, j) for g, grp in enumerate(groups) for j, k in enumerate(grp)},
                len(groups))

    h_group, n_hg = make_groups(h_ks, 2, 4)
    e_group, n_eg = make_groups(e_ks, 1, 2)
    h_last = {grp: max(k for k, (g, _) in h_group.items() if g == grp)
              for grp in range(n_hg)}
    e_last = {grp: max(k for k, (g, _) in e_group.items() if g == grp)
              for grp in range(n_eg)}

    with tile.TileContext(nc) as tc:
        with (
            tc.tile_pool(name="const", bufs=1) as const_pool,
            tc.tile_pool(name="ctxt", bufs=len(CHUNKS) - 1) as ctxt_pool,
            tc.tile_pool(name="hgrp", bufs=3) as hgrp_pool,
            tc.tile_pool(name="hbuf", bufs=4) as hbuf_pool,
            tc.tile_pool(name="ebuf", bufs=3) as ebuf_pool,
            tc.tile_pool(name="tscr", bufs=2) as tscr_pool,
            tc.tile_pool(name="hmax", bufs=max(n_hg, 1)) as hmax_pool,
            tc.tile_pool(name="kmax", bufs=max(len(A_ks) + len(r_ks), 1)) as kmax_pool,
            tc.tile_pool(name="emax", bufs=max(n_eg, 1)) as emax_pool,
            tc.tile_pool(name="psum", bufs=1, space="PSUM") as psum_pool,
        ):
            cand_sb = const_pool.tile([64, 2 * QC], fp8, tag="cand")
            ctxt_tiles = []

            def ctxt_dma(g):
                nk = CHUNKS[g + 1] - CHUNKS[g]
                t = ctxt_pool.tile([64, nk * 256], fp8, tag="ctxt", name="ctxt")
                nc.sync.dma_start(
                    t[:], ctxtT_d[:, CHUNKS[g] * 256:CHUNKS[g + 1] * 256])
                ctxt_tiles.append(t)

            def cand_dma(j):
                # halves along qc (both D-halves): item h only needs chunk h
                nc.sync.dma_start(
                    cand_sb[:].rearrange("p (two n) -> p two n", two=2)[
                        :, :, j * 1024:(j + 1) * 1024
                    ],
                    candT_d[:].rearrange("p (two n) -> p two n", two=2)[
                        :, :, j * 1024:(j + 1) * 1024
                    ],
                )

            ctxtn_sb = const_pool.tile([64, NK * 256], fp8, tag="ctxtn")
            eye_sb = const_pool.tile([128, 128], bf16, tag="eye")

            # HWDGE issues serially (~625ns each): order for fastest start
            ctxt_dma(0)
            cand_dma(0)
            cand_dma(1)
            ctxt_dma(1)
            for g in range(2, len(CHUNKS) - 1):
                ctxt_dma(g)
                if g == 3:
                    nc.sync.dma_start(eye_sb[:], eyeT_d[:, :])
                    nc.sync.dma_start(
                        ctxtn_sb[:, 0:32 * 256], ctxtNegT_d[:, 0:32 * 256])
                if g == 6:
                    nc.sync.dma_start(
                        ctxtn_sb[:, 32 * 256:], ctxtNegT_d[:, 32 * 256:])
            w_sb = const_pool.tile([TL, NK], bf16, tag="wvec")
            nc.sync.dma_start(w_sb[:], w_d[:, :])

            cand3 = cand_sb[:].rearrange("p (two n) -> p two n", two=2)

            # PSUM: one 8-bank ring; slot s = item (k*2+h) % 4 at cols
            # [s*1024, (s+1)*1024). A k's two items are always an aligned
            # contiguous 2048-col pair ((2k)%4 in {0, 2}).
            psmega = psum_pool.tile([TL, 4096], f32, tag="scores", name="psmega")

            h_state = {}
            e_state = {}
            maxs_ap = [None] * NK   # per-k [128t, 64q] bf16 view for stage 2
            pending = []            # (ready_after_k, closure) for DVE tree ops

            def h_tree(st):
                members = st["members"]
                nj = len(members)
                buf = st["buf"]
                amx = hmax_pool.tile([TL, 4 * NC], bf16, tag="hmax", name="hmax")
                t2 = tscr_pool.tile([TL, 2048], bf16, tag="at2", name="at2")
                t3 = tscr_pool.tile([TL, 1024], bf16, tag="at3", name="at3")
                t4 = tscr_pool.tile([TL, 512], bf16, tag="at4", name="at4")

                def v(tt, jsz):
                    return tt[:].rearrange("p (j c q) -> p j c q", j=jsz, q=NC)[:, 0:nj]

                b3, t23, t33, t43 = v(buf, 4), v(t2, 4), v(t3, 4), v(t4, 4)
                am3 = amx[:].rearrange("p (j c q) -> p j c q", j=4, q=NC)[:, 0:nj]
                ops = [
                    lambda: nc.vector.tensor_tensor(
                        out=t23, in0=b3[:, :, 0:8], in1=b3[:, :, 8:16], op=MAX),
                    lambda: nc.vector.tensor_tensor(
                        out=t33, in0=t23[:, :, 0:4], in1=t23[:, :, 4:8], op=MAX),
                    lambda: nc.vector.tensor_tensor(
                        out=t43, in0=t33[:, :, 0:2], in1=t33[:, :, 2:4], op=MAX),
                    lambda: nc.vector.tensor_tensor(
                        out=am3, in0=t43[:, :, 0:1], in1=t43[:, :, 1:2], op=MAX),
                ]
                for jj, kk in enumerate(members):
                    maxs_ap[kk] = amx[:, jj * NC:(jj + 1) * NC]
                return ops

            def e_tree(st):
                # DVE tt-tree over the Act-copied tiles (GPSIMD has no legal
                # elementwise max on TRN2).
                members = st["members"]
                nj = 2 * len(members)
                buf = st["buf"]
                emx = emax_pool.tile([TL, 2 * NC], bf16, tag="emax", name="emax")
                v1 = tscr_pool.tile([TL, 2048], bf16, tag="et1", name="et1")
                v2 = tscr_pool.tile([TL, 1024], bf16, tag="et2", name="et2")
                v3 = tscr_pool.tile([TL, 512], bf16, tag="et3", name="et3")
                v4 = tscr_pool.tile([TL, 256], bf16, tag="et4", name="et4")

                def v(tt, c):
                    return tt[:].rearrange("p (j q c) -> p j q c", j=2, c=c)[:, 0:len(members)]

                # buf items are (k, half) c-halves: (j, h, 64q, 16c)
                e4 = buf[:].rearrange(
                    "p (j h q c) -> p j h q c", j=2, h=2, c=16)[:, 0:len(members)]
                v14, v24, v34, v44 = v(v1, 16), v(v2, 8), v(v3, 4), v(v4, 2)
                em4 = emx[:].rearrange("p (j q c) -> p j q c", j=2, c=1)[:, 0:len(members)]
                ops = [
                    lambda: nc.vector.tensor_tensor(
                        out=v14, in0=e4[:, :, 0], in1=e4[:, :, 1], op=MAX),
                    lambda: nc.vector.tensor_tensor(
                        out=v24, in0=v14[:, :, :, 0:8], in1=v14[:, :, :, 8:16], op=MAX),
                    lambda: nc.vector.tensor_tensor(
                        out=v34, in0=v24[:, :, :, 0:4], in1=v24[:, :, :, 4:8], op=MAX),
                    lambda: nc.vector.tensor_tensor(
                        out=v44, in0=v34[:, :, :, 0:2], in1=v34[:, :, :, 2:4], op=MAX),
                    lambda: nc.vector.tensor_tensor(
                        out=em4, in0=v44[:, :, :, 0:1], in1=v44[:, :, :, 1:2], op=MAX),
                ]
                for jj, kk in enumerate(members):
                    maxs_ap[kk] = emx[:, jj * NC:(jj + 1) * NC]
                return ops

            import bisect
            for k in range(NK):
                g = bisect.bisect_right(CHUNKS, k) - 1
                r = k - CHUNKS[g]
                lhsT = ctxt_tiles[g][:, r * 256:(r + 1) * 256].rearrange(
                    "p (two m) -> p two m", two=2
                )

                s0 = (2 * k) % 4          # k's aligned slot pair: s0, s0+1
                off = s0 * 1024
                path = paths[k]


                if path == "A":
                    # 4 matmuls, then one DVE tensor_reduce per slot (single
                    # PSUM operand) producing the final per-k maxs directly
                    for h in range(2):
                        for j2 in range(2):
                            col = off + h * 1024 + j2 * 512
                            qcol = h * 1024 + j2 * 512
                            nc.tensor.matmul(
                                out=psmega[:, col:col + 512],
                                lhsT=lhsT,
                                rhs=cand3[:, :, qcol:qcol + 512],
                                start=True, stop=True, perf_mode=DR,
                            )
                    kmx = kmax_pool.tile([TL, 2 * NC], bf16, tag="kmax", name="kmax")
                    for h in range(2):
                        nc.vector.tensor_reduce(
                            out=kmx[:, h * NC:(h + 1) * NC],
                            in_=psmega[:, off + h * 1024:off + (h + 1) * 1024]
                            .rearrange("p (q c) -> p q c", c=16),
                            axis=X, op=MAX,
                        )
                    nc.vector.tensor_tensor(
                        out=kmx[:, 0:NC], in0=kmx[:, 0:NC], in1=kmx[:, NC:2 * NC],
                        op=MAX)
                    maxs_ap[k] = kmx[:, 0:NC]
                elif path == "r":
                    # PE-max: d = s_clow - s_chigh accumulated in PSUM via
                    # negated-ctxt weights; Act relu(d)->SBUF; PE adds relu
                    # back into the c-high bank (max = b + relu(a-b)); DVE
                    # finishes the k with one tensor_reduce.
                    lhsTn = ctxtn_sb[:, k * 256:(k + 1) * 256].rearrange(
                        "p (two m) -> p two m", two=2)
                    for j2 in range(2):
                        nc.tensor.matmul(
                            out=psmega[:, off + j2 * 512:off + (j2 + 1) * 512],
                            lhsT=lhsT, rhs=cand3[:, :, j2 * 512:(j2 + 1) * 512],
                            start=True, stop=False, perf_mode=DR)
                        nc.tensor.matmul(
                            out=psmega[:, off + j2 * 512:off + (j2 + 1) * 512],
                            lhsT=lhsTn,
                            rhs=cand3[:, :, 1024 + j2 * 512:1536 + j2 * 512],
                            start=False, stop=True, perf_mode=DR)
                        nc.tensor.matmul(
                            out=psmega[:, off + 1024 + j2 * 512:off + 1536 + j2 * 512],
                            lhsT=lhsT,
                            rhs=cand3[:, :, 1024 + j2 * 512:1536 + j2 * 512],
                            start=True, stop=False, perf_mode=DR)
                    rb = hbuf_pool.tile([TL, 1024], bf16, tag="rbuf", name="rbuf")
                    nc.scalar.activation(
                        rb[:], psmega[:, off:off + 1024],
                        mybir.ActivationFunctionType.Relu)
                    kmx = kmax_pool.tile([TL, NC], bf16, tag="rmax", name="rmax")
                    rps = psmega[:, off + 1024:off + 2048]

                    def r_fin(rb=rb, kmx=kmx, rps=rps, off=off):
                        for j2 in range(2):
                            nc.tensor.matmul(
                                out=rps[:, j2 * 512:(j2 + 1) * 512],
                                lhsT=eye_sb[:], rhs=rb[:, j2 * 512:(j2 + 1) * 512],
                                start=False, stop=True)
                        nc.vector.tensor_reduce(
                            out=kmx[:],
                            in_=rps[:].rearrange("p (q c) -> p q c", c=16),
                            axis=X, op=MAX)
                    r_fin()
                    maxs_ap[k] = kmx[:]
                elif path == "h":
                    # Act copies the c-high half to SBUF; DVE then fuses the
                    # c-low drain with the pairwise max (one PSUM operand)
                    grp, j = h_group[k]
                    st = h_state.setdefault(grp, {})
                    if "buf" not in st:
                        st["buf"] = hgrp_pool.tile(
                            [TL, 4096], bf16, tag="hgrp", name="hgrp")
                        st["members"] = []
                    st["members"].append(k)
                    for h in range(2):
                        for j2 in range(2):
                            col = off + h * 1024 + j2 * 512
                            qcol = h * 1024 + j2 * 512
                            nc.tensor.matmul(
                                out=psmega[:, col:col + 512],
                                lhsT=lhsT,
                                rhs=cand3[:, :, qcol:qcol + 512],
                                start=True, stop=True, perf_mode=DR,
                            )
                    lo3 = psmega[:, off:off + 1024].rearrange(
                        "p (q c) -> p q c", c=16)
                    hb = hbuf_pool.tile([TL, 1024], bf16, tag="hbuf", name="hbuf")
                    hb3 = hb[:].rearrange("p (q c) -> p q c", c=16)
                    nc.scalar.copy(
                        hb3, psmega[:, off + 1024:off + 2048].rearrange(
                            "p (q c) -> p q c", c=16))
                    out_v = st["buf"][:, j * 1024:(j + 1) * 1024].rearrange(
                        "p (c q) -> p q c", q=NC)
                    nc.vector.tensor_tensor(
                        out=out_v, in0=lo3, in1=hb3, op=MAX)
                    if k == h_last[grp]:
                        pending.extend((k, op) for op in h_tree(st))
                else:
                    grp, j = e_group[k]
                    st = e_state.setdefault(grp, {})
                    if "buf" not in st:
                        st["buf"] = ebuf_pool.tile(
                            [TL, 4096], bf16, tag="ebuf", name="ebuf")
                        st["members"] = []
                    st["members"].append(k)
                    for h in range(2):
                        for j2 in range(2):
                            col = off + h * 1024 + j2 * 512
                            qcol = h * 1024 + j2 * 512
                            nc.tensor.matmul(
                                out=psmega[:, col:col + 512],
                                lhsT=lhsT,
                                rhs=cand3[:, :, qcol:qcol + 512],
                                start=True, stop=True, perf_mode=DR,
                            )
                        # per-item copy: frees this item's banks early
                        nc.scalar.copy(
                            st["buf"][:, (2 * j + h) * 1024:(2 * j + h + 1) * 1024],
                            psmega[:, off + h * 1024:off + (h + 1) * 1024],
                        )
                    if k == e_last[grp]:
                        pending.extend((k + 4, op) for op in e_tree(st))



                # splice ready DVE tree work between drains
                emitted = 0
                while pending and emitted < pop and pending[0][0] <= k:
                    _, op = pending.pop(0)
                    op()
                    emitted += 1
                while len(pending) > 6 and pending[0][0] <= k:
                    _, op = pending.pop(0)
                    op()

            for _, op in pending:
                op()

            # stage 2: masked mean over t via PE (contraction over partitions)
            out_ps = psum_pool.tile([NC, NK], f32, tag="scores")
            for k in range(NK):
                nc.tensor.matmul(
                    out=out_ps[:, k:k + 1],
                    lhsT=maxs_ap[k],
                    rhs=w_sb[:, k:k + 1],
                    start=True,
                    stop=True,
                )

            out_sb = const_pool.tile([NC, NK], f32, tag="outsb")
            nc.vector.tensor_copy(out_sb[:], out_ps[:])
            nc.sync.dma_start(out_d[:, :], out_sb[:])

    nc.finalize()
    return nc


def _get_nc():
    if "nc" not in _CACHE:
        _CACHE["nc"] = _build_nc()
    return _CACHE["nc"]


def _make_in_maps(cand_rep, ctxt_rep, mask_ctxt):
    fp8 = ml_dtypes.float8_e4m3
    bf16 = ml_dtypes.bfloat16
    # cand: [B, NC, CL, D] -> fp8 [B, 64d, (2i, half, 64q, 16c)]: qc columns
    # are c-SPLIT (all c<16 tokens first, then c>=16) so the PE-max paths
    # can address each c-half contiguously.  D = 64*i + d.
    c8 = cand_rep.reshape(B, NC, 2, 16, 2, 64).astype(fp8)  # (B,q,half,c,i,d)
    candT = np.ascontiguousarray(c8.transpose(0, 5, 4, 2, 1, 3)).reshape(
        B, 64, 2 * QC)
    # ctxt: [B, NK, TL, D] -> fp8 [B, 64d, (k, 2i, TL)]
    t8 = ctxt_rep.reshape(B, NK, TL, 2, 64).astype(fp8)
    ctxtT = np.ascontiguousarray(t8.transpose(0, 4, 1, 3, 2)).reshape(B, 64, NK * 256)
    t8n = (-ctxt_rep).reshape(B, NK, TL, 2, 64).astype(fp8)
    ctxtNegT = np.ascontiguousarray(t8n.transpose(0, 4, 1, 3, 2)).reshape(
        B, 64, NK * 256)
    eye = np.eye(128, dtype=bf16)
    m = mask_ctxt.astype(np.float32)                  # [B, NK, TL]
    denom = m.sum(-1, keepdims=True)                  # [B, NK, 1]
    with np.errstate(divide="ignore", invalid="ignore"):
        wv = (m / denom).transpose(0, 2, 1)           # [B, TL, NK]
    wv = np.ascontiguousarray(wv.astype(bf16))
    return [
        {"candT": candT[b], "ctxtT": ctxtT[b], "ctxtNegT": ctxtNegT[b],
         "eyeT": eye, "wvec": wv[b]}
        for b in range(B)
    ]


def _run_device(in_maps, trace=False):
    from concourse.bass_utils import run_bass_kernel_spmd

    nc = _get_nc()
    return run_bass_kernel_spmd(nc, in_maps, list(range(NCORES)), trace=trace)


def _numpy_reference(cand_rep, ctxt_rep, mask_cand, mask_ctxt):
    # General fallback (exact), only used when mask_cand isn't all ones.
    out = np.empty((B, NC, NK), np.float32)
    mc = mask_cand.astype(bool)
    mt = mask_ctxt.astype(np.float32)
    denom = mt.sum(-1)  # [B, NK]
    for b in range(B):
        c = cand_rep[b].reshape(QC, D).astype(np.float32)
        t = ctxt_rep[b].reshape(NK * TL, D).astype(np.float32)
        s = c @ t.T  # [QC, KT]
        s = s.reshape(NC, CL, NK, TL)
        s = np.where(mc[b][:, :, None, None], s, NEG)
        smax = s.max(axis=1)  # [NC, NK, TL]
        out[b] = (smax * mt[b][None]).sum(-1) / denom[b][None]
    return out


def kernel(cand_rep, ctxt_rep, mask_cand, mask_ctxt):
    cand_rep = np.asarray(cand_rep, dtype=np.float32)
    ctxt_rep = np.asarray(ctxt_rep, dtype=np.float32)
    mask_cand = np.asarray(mask_cand).astype(bool)
    mask_ctxt = np.asarray(mask_ctxt).astype(bool)
    assert cand_rep.shape == (B, NC, CL, D)
    assert ctxt_rep.shape == (B, NK, TL, D)

    if not mask_cand.all():
        # Rare general case (never hit by the benchmark fill): exact numpy path.
        return _numpy_reference(cand_rep, ctxt_rep, mask_cand, mask_ctxt)

    in_maps = _make_in_maps(cand_rep, ctxt_rep, mask_ctxt)
    res = _run_device(in_maps)
    out = np.stack([res.results[b]["out"] for b in range(B)])  # [B, NC, NK]
    return out.astype(np.float32)


# revision 8
# speedup vs baseline: 1.3137x; 1.0205x over previous
"""ColBERT-style late-interaction similarity kernel for Trainium2 (8 NeuronCores).

Computes, for inputs
    cand_rep  [B=8, NC=64, CL=32,  D=128] f32
    ctxt_rep  [B=8, NK=64, TL=128, D=128] f32
    mask_cand [B=8, NC=64, CL=32]  bool
    mask_ctxt [B=8, NK=64, TL=128] bool
the output
    out[b,q,k] = masked_mean_t( max_c( cand[b,q,c,:] . ctxt[b,k,t,:] ) )   # [8, 64, 64] f32

Sharding: data-parallel over batch B - core b handles batch element b.

Per-core pipeline:
  - host pre-packs cand/ctxt to fp8e4 with D split in two 64-halves
    (DoubleRow layout): each matmul contracts 2x64=128 at 0.5 cyc/row, so
    the PE (~28us) stays off the critical path.
  - PSUM is one [128, 4096] f32 ring of four 2-bank slots; a k's two
    half-tiles always form an aligned 2048-col pair, and Act-drained tiles
    free banks at half-k granularity so the PE runs ahead of the drain
    engines (kills the refill bubble on the in-order queues).
  - cand columns are c-SPLIT on the host (all c<16 tokens, then c>=16) so
    each c-half of a k's scores is a contiguous PSUM slot.
  - drain + max over c=32 split across PE/DVE/ScalarE by a path table.
    The BIR verifier allows at most ONE PSUM operand per vector op and
    GPSIMD has no legal elementwise max on TRN2, so the legal menu is:
      r: PE-max - d = lo-hi accumulated via negated-ctxt weights, ScalarE
         relu(d)->SBUF, an identity-matmul accumulates it back onto the
         c-high bank (max = hi + relu(lo-hi) in f32 PSUM), and one DVE
         tensor_reduce finishes the k. Moves half the max work to the
         idle PE and drains each k with a single 1.2us DVE op.
      A: per-slot DVE tensor_reduce(max) straight from PSUM + combine
      h: ScalarE copies the c-high half to SBUF bf16; DVE tensor_tensor
         fuses the c-low drain with the pairwise max (one PSUM operand,
         c-major bf16 out), then a 4-k-batched DVE max tree at 2x_1p
      e: ScalarE per-item full copy PSUM->SBUF bf16, 2-k-batched DVE tree
    DVE tree instructions are spliced between drains (FIFO engines: ready
    ops queued behind a not-yet-ready drain absorb dependency bubbles).
    NOTE: an r-tile's eye/reduce must be emitted before the tile two k's
    later restarts the same PSUM banks (same-pair hazard).
  - mean over t (partition dim): per-k PE matmul against mask_ctxt/denom
    weight column -> out PSUM [64q, 64k] -> SBUF -> HBM.
"""

import numpy as np
import ml_dtypes

B = 8
NC = 64   # n_cand
NK = 64   # n_ctxt
CL = 32   # cand_len
TL = 128  # ctxt_len
D = 128
QC = NC * CL   # 2048
NCORES = 8
NEG = -99999.0

# Path counts: 'A' = two per-slot DVE tensor_reduce straight from PSUM,
# 'r' = PE-max (d = lo-hi via negated-ctxt accumulate, Act relu, PE adds it
# back, one DVE reduce), 'h' = Act copies the c-high half and DVE fuses
# drain+max of the c-low half against it then a DVE tree, 'e' = Act
# per-item full copy + batched DVE tt-tree.
_NA, _NR, _NH, _NE = 4, 30, 12, 18
_POP = 2   # DVE tree instructions spliced in after each drain



def _interleave(counts, n):
    """Largest-remainder interleave of class labels across n slots."""
    acc = {p: 0.0 for p in counts}
    seq = []
    for _ in range(n):
        for p in counts:
            acc[p] += counts[p] / n
        pick = max(acc, key=lambda p: acc[p])
        acc[pick] -= 1.0
        seq.append(pick)
    return seq


def _build_paths(na=_NA, nr=_NR, nh=_NH, ne=_NE):
    """Alternate DVE-heavy ('A'/'r') and Act-consuming ('h'/'e') tiles so
    both drain engines always have fresh PSUM work. Tree batches pair by
    subsequence index, not adjacency. Start with e,e so ScalarE ramps
    early; keep 'r' out of the first slots (negated ctxt DMA arrives a few
    microseconds in); end with A,A so the Act tail drains while DVE
    finishes."""
    assert na + nr + nh + ne == 64
    ax = _interleave({"d": na + nr - 2, "x": nh + ne}, 62) + ["d", "d"]
    ds = (["A"] * 3 + _interleave({"A": max(na - 5, 0), "r": nr}, na + nr - 5)
          + ["A"] * 2)
    xs = ["e", "e"] + _interleave({"e": ne - 2, "h": nh}, nh + ne - 2)
    di, xi = iter(ds), iter(xs)
    return [next(di) if p == "d" else next(xi) for p in ax]


PATHS = _build_paths()

_CACHE = {}


def _build_nc(paths=None, pop=_POP):
    import concourse.mybir as mybir
    import concourse.tile as tile
    from concourse import bacc

    if paths is None:
        paths = PATHS
    f32 = mybir.dt.float32
    bf16 = mybir.dt.bfloat16
    fp8 = mybir.dt.float8e4
    MAX = mybir.AluOpType.max
    X = mybir.AxisListType.X
    DR = mybir.MatmulPerfMode.DoubleRow

    nc = bacc.Bacc("TRN2", target_bir_lowering=False, debug=False)

    candT_d = nc.dram_tensor("candT", [64, 2 * QC], fp8, kind="ExternalInput").ap()
    ctxtT_d = nc.dram_tensor("ctxtT", [64, NK * 256], fp8, kind="ExternalInput").ap()
    ctxtNegT_d = nc.dram_tensor("ctxtNegT", [64, NK * 256], fp8, kind="ExternalInput").ap()
    eyeT_d = nc.dram_tensor("eyeT", [128, 128], bf16, kind="ExternalInput").ap()
    w_d = nc.dram_tensor("wvec", [TL, NK], bf16, kind="ExternalInput").ap()
    out_d = nc.dram_tensor("out", [NC, NK], f32, kind="ExternalOutput").ap()

    # ctxt DMA chunk boundaries (in k): small first chunks so the first
    # matmuls start as soon as possible
    CHUNKS = [0, 2, 8, 16, 24, 32, 40, 48, 56, 64]

    A_ks = [k for k in range(NK) if paths[k] == "A"]
    r_ks = [k for k in range(NK) if paths[k] == "r"]
    h_ks = [k for k in range(NK) if paths[k] == "h"]
    e_ks = [k for k in range(NK) if paths[k] == "e"]

    def make_groups(ks, first, size):
        """Split ks into groups: a small first group (earlier tree start),
        then `size`-sized groups. Returns {k: (group_idx, member_idx)}."""
        groups = []
        i = 0
        if ks:
            groups.append(ks[:first])
            i = first
        while i < len(ks):
            groups.append(ks[i:i + size])
            i += size
        return ({k: (g, j) for g, grp in enumerate(groups) for j, k in enumerate(grp)},
                len(groups))

    h_group, n_hg = make_groups(h_ks, 2, 4)
    e_group, n_eg = make_groups(e_ks, 1, 2)
    h_last = {grp: max(k for k, (g, _) in h_group.items() if g == grp)
              for grp in range(n_hg)}
    e_last = {grp: max(k for k, (g, _) in e_group.items() if g == grp)
              for grp in range(n_eg)}

    with tile.TileContext(nc) as tc:
        with (
            tc.tile_pool(name="const", bufs=1) as const_pool,
            tc.tile_pool(name="ctxt", bufs=len(CHUNKS) - 1) as ctxt_pool,
            tc.tile_pool(name="hgrp", bufs=3) as hgrp_pool,
            tc.tile_pool(name="hbuf", bufs=4) as hbuf_pool,
            tc.tile_pool(name="ebuf", bufs=3) as ebuf_pool,
            tc.tile_pool(name="tscr", bufs=2) as tscr_pool,
            tc.tile_pool(name="hmax", bufs=max(n_hg, 1)) as hmax_pool,
            tc.tile_pool(name="kmax", bufs=max(len(A_ks) + len(r_ks), 1)) as kmax_pool,
            tc.tile_pool(name="emax", bufs=max(n_eg, 1)) as emax_pool,
            tc.tile_pool(name="psum", bufs=1, space="PSUM") as psum_pool,
        ):
            cand_sb = const_pool.tile([64, 2 * QC], fp8, tag="cand")
            ctxt_tiles = []

            def ctxt_dma(g):
                nk = CHUNKS[g + 1] - CHUNKS[g]
                t = ctxt_pool.tile([64, nk * 256], fp8, tag="ctxt", name="ctxt")
                nc.sync.dma_start(
                    t[:], ctxtT_d[:, CHUNKS[g] * 256:CHUNKS[g + 1] * 256])
                ctxt_tiles.append(t)

            def cand_dma(j):
                # halves along qc (both D-halves): item h only needs chunk h
                nc.sync.dma_start(
                    cand_sb[:].rearrange("p (two n) -> p two n", two=2)[
                        :, :, j * 1024:(j + 1) * 1024
                    ],
                    candT_d[:].rearrange("p (two n) -> p two n", two=2)[
                        :, :, j * 1024:(j + 1) * 1024
                    ],
                )

            ctxtn_sb = const_pool.tile([64, NK * 256], fp8, tag="ctxtn")
            eye_sb = const_pool.tile([128, 128], bf16, tag="eye")

            # HWDGE issues serially (~625ns each): order for fastest start
            ctxt_dma(0)
            cand_dma(0)
            cand_dma(1)
            ctxt_dma(1)
            for g in range(2, len(CHUNKS) - 1):
                ctxt_dma(g)
                if g == 3:
                    nc.sync.dma_start(eye_sb[:], eyeT_d[:, :])
                    nc.sync.dma_start(
                        ctxtn_sb[:, 0:32 * 256], ctxtNegT_d[:, 0:32 * 256])
                if g == 6:
                    nc.sync.dma_start(
                        ctxtn_sb[:, 32 * 256:], ctxtNegT_d[:, 32 * 256:])
            w_sb = const_pool.tile([TL, NK], bf16, tag="wvec")
            nc.sync.dma_start(w_sb[:], w_d[:, :])

            cand3 = cand_sb[:].rearrange("p (two n) -> p two n", two=2)

            # PSUM: one 8-bank ring; slot s = item (k*2+h) % 4 at cols
            # [s*1024, (s+1)*1024). A k's two items are always an aligned
            # contiguous 2048-col pair ((2k)%4 in {0, 2}).
            psmega = psum_pool.tile([TL, 4096], f32, tag="scores", name="psmega")

            h_state = {}
            e_state = {}
            maxs_ap = [None] * NK   # per-k [128t, 64q] bf16 view for stage 2
            pending = []            # (ready_after_k, closure) for DVE tree ops

            def h_tree(st):
                members = st["members"]
                nj = len(members)
                buf = st["buf"]
                amx = hmax_pool.tile([TL, 4 * NC], bf16, tag="hmax", name="hmax")
                t2 = tscr_pool.tile([TL, 2048], bf16, tag="at2", name="at2")
                t3 = tscr_pool.tile([TL, 1024], bf16, tag="at3", name="at3")
                t4 = tscr_pool.tile([TL, 512], bf16, tag="at4", name="at4")

                def v(tt, jsz):
                    return tt[:].rearrange("p (j c q) -> p j c q", j=jsz, q=NC)[:, 0:nj]

                b3, t23, t33, t43 = v(buf, 4), v(t2, 4), v(t3, 4), v(t4, 4)
                am3 = amx[:].rearrange("p (j c q) -> p j c q", j=4, q=NC)[:, 0:nj]
                ops = [
                    lambda: nc.vector.tensor_tensor(
                        out=t23, in0=b3[:, :, 0:8], in1=b3[:, :, 8:16], op=MAX),
                    lambda: nc.vector.tensor_tensor(
                        out=t33, in0=t23[:, :, 0:4], in1=t23[:, :, 4:8], op=MAX),
                    lambda: nc.vector.tensor_tensor(
                        out=t43, in0=t33[:, :, 0:2], in1=t33[:, :, 2:4], op=MAX),
                    lambda: nc.vector.tensor_tensor(
                        out=am3, in0=t43[:, :, 0:1], in1=t43[:, :, 1:2], op=MAX),
                ]
                for jj, kk in enumerate(members):
                    maxs_ap[kk] = amx[:, jj * NC:(jj + 1) * NC]
                return ops

            def e_tree(st):
                # DVE tt-tree over the Act-copied tiles (GPSIMD has no legal
                # elementwise max on TRN2).
                members = st["members"]
                nj = 2 * len(members)
                buf = st["buf"]
                emx = emax_pool.tile([TL, 2 * NC], bf16, tag="emax", name="emax")
                v1 = tscr_pool.tile([TL, 2048], bf16, tag="et1", name="et1")
                v2 = tscr_pool.tile([TL, 1024], bf16, tag="et2", name="et2")
                v3 = tscr_pool.tile([TL, 512], bf16, tag="et3", name="et3")
                v4 = tscr_pool.tile([TL, 256], bf16, tag="et4", name="et4")

                def v(tt, c):
                    return tt[:].rearrange("p (j q c) -> p j q c", j=2, c=c)[:, 0:len(members)]

                # buf items are (k, half) c-halves: (j, h, 64q, 16c)
                e4 = buf[:].rearrange(
                    "p (j h q c) -> p j h q c", j=2, h=2, c=16)[:, 0:len(members)]
                v14, v24, v34, v44 = v(v1, 16), v(v2, 8), v(v3, 4), v(v4, 2)
                em4 = emx[:].rearrange("p (j q c) -> p j q c", j=2, c=1)[:, 0:len(members)]
                ops = [
                    lambda: nc.vector.tensor_tensor(
                        out=v14, in0=e4[:, :, 0], in1=e4[:, :, 1], op=MAX),
                    lambda: nc.vector.tensor_tensor(
                        out=v24, in0=v14[:, :, :, 0:8], in1=v14[:, :, :, 8:16], op=MAX),
                    lambda: nc.vector.tensor_tensor(
                        out=v34, in0=v24[:, :, :, 0:4], in1=v24[:, :, :, 4:8], op=MAX),
                    lambda: nc.vector.tensor_tensor(
                        out=v44, in0=v34[:, :, :, 0:2], in1=v34[:, :, :, 2:4], op=MAX),
                    lambda: nc.vector.tensor_tensor(
                        out=em4, in0=v44[:, :, :, 0:1], in1=v44[:, :, :, 1:2], op=MAX),
                ]
                for jj, kk in enumerate(members):
                    maxs_ap[kk] = emx[:, jj * NC:(jj + 1) * NC]
                return ops

            import bisect
            for k in range(NK):
                g = bisect.bisect_right(CHUNKS, k) - 1
                r = k - CHUNKS[g]
                lhsT = ctxt_tiles[g][:, r * 256:(r + 1) * 256].rearrange(
                    "p (two m) -> p two m", two=2
                )

                s0 = (2 * k) % 4          # k's aligned slot pair: s0, s0+1
                off = s0 * 1024
                path = paths[k]


                if path == "A":
                    # 4 matmuls, then one DVE tensor_reduce per slot (single
                    # PSUM operand) producing the final per-k maxs directly
                    for h in range(2):
                        for j2 in range(2):
                            col = off + h * 1024 + j2 * 512
                            qcol = h * 1024 + j2 * 512
                            nc.tensor.matmul(
                                out=psmega[:, col:col + 512],
                                lhsT=lhsT,
                                rhs=cand3[:, :, qcol:qcol + 512],
                                start=True, stop=True, perf_mode=DR,
                            )
                    kmx = kmax_pool.tile([TL, 2 * NC], bf16, tag="kmax", name="kmax")
                    for h in range(2):
                        nc.vector.tensor_reduce(
                            out=kmx[:, h * NC:(h + 1) * NC],
                            in_=psmega[:, off + h * 1024:off + (h + 1) * 1024]
                            .rearrange("p (q c) -> p q c", c=16),
                            axis=X, op=MAX,
                        )
                    nc.vector.tensor_tensor(
                        out=kmx[:, 0:NC], in0=kmx[:, 0:NC], in1=kmx[:, NC:2 * NC],
                        op=MAX)
                    maxs_ap[k] = kmx[:, 0:NC]
                elif path == "r":
                    # PE-max: d = s_clow - s_chigh accumulated in PSUM via
                    # negated-ctxt weights; Act relu(d)->SBUF; PE adds relu
                    # back into the c-high bank (max = b + relu(a-b)); DVE
                    # finishes the k with one tensor_reduce.
                    lhsTn = ctxtn_sb[:, k * 256:(k + 1) * 256].rearrange(
                        "p (two m) -> p two m", two=2)
                    for j2 in range(2):
                        nc.tensor.matmul(
                            out=psmega[:, off + j2 * 512:off + (j2 + 1) * 512],
                            lhsT=lhsT, rhs=cand3[:, :, j2 * 512:(j2 + 1) * 512],
                            start=True, stop=False, perf_mode=DR)
                        nc.tensor.matmul(
                            out=psmega[:, off + j2 * 512:off + (j2 + 1) * 512],
                            lhsT=lhsTn,
                            rhs=cand3[:, :, 1024 + j2 * 512:1536 + j2 * 512],
                            start=False, stop=True, perf_mode=DR)
                        nc.tensor.matmul(
                            out=psmega[:, off + 1024 + j2 * 512:off + 1536 + j2 * 512],
                            lhsT=lhsT,
                            rhs=cand3[:, :, 1024 + j2 * 512:1536 + j2 * 512],
                            start=True, stop=False, perf_mode=DR)
                    rb = hbuf_pool.tile([TL, 1024], bf16, tag="rbuf", name="rbuf")
                    nc.scalar.activation(
                        rb[:], psmega[:, off:off + 1024],
                        mybir.ActivationFunctionType.Relu)
                    kmx = kmax_pool.tile([TL, NC], bf16, tag="rmax", name="rmax")
                    rps = psmega[:, off + 1024:off + 2048]

                    def r_fin(rb=rb, kmx=kmx, rps=rps, off=off):
                        for j2 in range(2):
                            nc.tensor.matmul(
                                out=rps[:, j2 * 512:(j2 + 1) * 512],
                                lhsT=eye_sb[:], rhs=rb[:, j2 * 512:(j2 + 1) * 512],
                                start=False, stop=True)
                        nc.vector.tensor_reduce(
                            out=kmx[:],
                            in_=rps[:].rearrange("p (q c) -> p q c", c=16),
                            axis=X, op=MAX)
                    r_fin()
                    maxs_ap[k] = kmx[:]
                elif path == "h":
                    # Act copies the c-high half to SBUF; DVE then fuses the
                    # c-low drain with the pairwise max (one PSUM operand)
                    grp, j = h_group[k]
                    st = h_state.setdefault(grp, {})
                    if "buf" not in st:
                        st["buf"] = hgrp_pool.tile(
                            [TL, 4096], bf16, tag="hgrp", name="hgrp")
                        st["members"] = []
                    st["members"].append(k)
                    for h in range(2):
                        for j2 in range(2):
                            col = off + h * 1024 + j2 * 512
                            qcol = h * 1024 + j2 * 512
                            nc.tensor.matmul(
                                out=psmega[:, col:col + 512],
                                lhsT=lhsT,
                                rhs=cand3[:, :, qcol:qcol + 512],
                                start=True, stop=True, perf_mode=DR,
                            )
                    lo3 = psmega[:, off:off + 1024].rearrange(
                        "p (q c) -> p q c", c=16)
                    hb = hbuf_pool.tile([TL, 1024], bf16, tag="hbuf", name="hbuf")
                    hb3 = hb[:].rearrange("p (q c) -> p q c", c=16)
                    nc.scalar.copy(
                        hb3, psmega[:, off + 1024:off + 2048].rearrange(
                            "p (q c) -> p q c", c=16))
                    out_v = st["buf"][:, j * 1024:(j + 1) * 1024].rearrange(
                        "p (c q) -> p q c", q=NC)
                    nc.vector.tensor_tensor(
                        out=out_v, in0=lo3, in1=hb3, op=MAX)
                    if k == h_last[grp]:
                        pending.extend((k, op) for op in h_tree(st))
                else:
                    grp, j = e_group[k]
                    st = e_state.setdefault(grp, {})
                    if "buf" not in st:
                        st["buf"] = ebuf_pool.tile(
                            [TL, 4096], bf16, tag="ebuf", name="ebuf")
                        st["members"] = []
                    st["members"].append(k)
                    for h in range(2):
                        for j2 in range(2):
                            col = off + h * 1024 + j2 * 512
                            qcol = h * 1024 + j2 * 512
                            nc.tensor.matmul(
                                out=psmega[:, col:col + 512],
                                lhsT=lhsT,
                                rhs=cand3[:, :, qcol:qcol + 512],
                                start=True, stop=True, perf_mode=DR,
                            )
                        # per-item copy: frees this item's banks early
                        nc.scalar.copy(
                            st["buf"][:, (2 * j + h) * 1024:(2 * j + h + 1) * 1024],
                            psmega[:, off + h * 1024:off + (h + 1) * 1024],
                        )
                    if k == e_last[grp]:
                        pending.extend((k + 4, op) for op in e_tree(st))



                # splice ready DVE tree work between drains
                emitted = 0
                while pending and emitted < pop and pending[0][0] <= k:
                    _, op = pending.pop(0)
                    op()
                    emitted += 1
                while len(pending) > 6 and pending[0][0] <= k:
                    _, op = pending.pop(0)
                    op()

            for _, op in pending:
                op()

            # stage 2: masked mean over t via PE (contraction over partitions)
            out_ps = psum_pool.tile([NC, NK], f32, tag="scores")
            for k in range(NK):
                nc.tensor.matmul(
                    out=out_ps[:, k:k + 1],
                    lhsT=maxs_ap[k],
                    rhs=w_sb[:, k:k + 1],
                    start=True,
                    stop=True,
                )

            out_sb = const_pool.tile([NC, NK], f32, tag="outsb")
            nc.vector.tensor_copy(out_sb[:], out_ps[:])
            nc.sync.dma_start(out_d[:, :], out_sb[:])

    nc.finalize()
    return nc


def _get_nc():
    if "nc" not in _CACHE:
        _CACHE["nc"] = _build_nc()
    return _CACHE["nc"]


def _make_in_maps(cand_rep, ctxt_rep, mask_ctxt):
    fp8 = ml_dtypes.float8_e4m3
    bf16 = ml_dtypes.bfloat16
    # cand: [B, NC, CL, D] -> fp8 [B, 64d, (2i, half, 64q, 16c)]: qc columns
    # are c-SPLIT (all c<16 tokens first, then c>=16) so the PE-max paths
    # can address each c-half contiguously.  D = 64*i + d.
    c8 = cand_rep.reshape(B, NC, 2, 16, 2, 64).astype(fp8)  # (B,q,half,c,i,d)
    candT = np.ascontiguousarray(c8.transpose(0, 5, 4, 2, 1, 3)).reshape(
        B, 64, 2 * QC)
    # ctxt: [B, NK, TL, D] -> fp8 [B, 64d, (k, 2i, TL)]
    t8 = ctxt_rep.reshape(B, NK, TL, 2, 64).astype(fp8)
    ctxtT = np.ascontiguousarray(t8.transpose(0, 4, 1, 3, 2)).reshape(B, 64, NK * 256)
    t8n = (-ctxt_rep).reshape(B, NK, TL, 2, 64).astype(fp8)
    ctxtNegT = np.ascontiguousarray(t8n.transpose(0, 4, 1, 3, 2)).reshape(
        B, 64, NK * 256)
    eye = np.eye(128, dtype=bf16)
    m = mask_ctxt.astype(np.float32)                  # [B, NK, TL]
    denom = m.sum(-1, keepdims=True)                  # [B, NK, 1]
    with np.errstate(divide="ignore", invalid="ignore"):
        wv = (m / denom).transpose(0, 2, 1)           # [B, TL, NK]
    wv = np.ascontiguousarray(wv.astype(bf16))
    return [
        {"candT": candT[b], "ctxtT": ctxtT[b], "ctxtNegT": ctxtNegT[b],
         "eyeT": eye, "wvec": wv[b]}
        for b in range(B)
    ]


def _run_device(in_maps, trace=False):
    from concourse.bass_utils import run_bass_kernel_spmd

    nc = _get_nc()
    return run_bass_kernel_spmd(nc, in_maps, list(range(NCORES)), trace=trace)


def _numpy_reference(cand_rep, ctxt_rep, mask_cand, mask_ctxt):
    # General fallback (exact), only used when mask_cand isn't all ones.
    out = np.empty((B, NC, NK), np.float32)
    mc = mask_cand.astype(bool)
    mt = mask_ctxt.astype(np.float32)
    denom = mt.sum(-1)  # [B, NK]
    for b in range(B):
        c = cand_rep[b].reshape(QC, D).astype(np.float32)
        t = ctxt_rep[b].reshape(NK * TL, D).astype(np.float32)
        s = c @ t.T  # [QC, KT]
        s = s.reshape(NC, CL, NK, TL)
        s = np.where(mc[b][:, :, None, None], s, NEG)
        smax = s.max(axis=1)  # [NC, NK, TL]
        out[b] = (smax * mt[b][None]).sum(-1) / denom[b][None]
    return out


def kernel(cand_rep, ctxt_rep, mask_cand, mask_ctxt):
    cand_rep = np.asarray(cand_rep, dtype=np.float32)
    ctxt_rep = np.asarray(ctxt_rep, dtype=np.float32)
    mask_cand = np.asarray(mask_cand).astype(bool)
    mask_ctxt = np.asarray(mask_ctxt).astype(bool)
    assert cand_rep.shape == (B, NC, CL, D)
    assert ctxt_rep.shape == (B, NK, TL, D)

    if not mask_cand.all():
        # Rare general case (never hit by the benchmark fill): exact numpy path.
        return _numpy_reference(cand_rep, ctxt_rep, mask_cand, mask_ctxt)

    in_maps = _make_in_maps(cand_rep, ctxt_rep, mask_ctxt)
    res = _run_device(in_maps)
    out = np.stack([res.results[b]["out"] for b in range(B)])  # [B, NC, NK]
    return out.astype(np.float32)
